# revision 1
# baseline (speedup 1.0000x reference)
"""Trainium2 Bass kernel for nn_EstimatorQNNGen104 (dense tiny-MLP over 4.2M rows).

Strategy (pure data parallel over 8 NeuronCores):
  - Shard batch across cores: R_core = B/8 = 524288 rows/core.
  - Host packs each core's (R_core, 4) input into a feature-banded layout:
    supertile ST = 8192 rows = 16 groups x 512 cols; band f occupies
    partitions [16f, 16f+16) (+64 for the "hi" ST of a pair).
  - All matmuls are full-width (128,128) float32r (TF32-class, 1 cyc/row)
    with block-diagonal lhsT matrices built on the host; PSUM outputs always
    start at partition 0; narrow outputs stack into shared banks via
    zero-column lhsT padding + PSUM accumulation.
  - Activations (tanh/sigmoid) run on ScalarE with per-partition bias APs
    (all layer biases folded there); softmax(2) is computed as
    s0 = sigmoid(d), s1 = 1-s0 folded into the next layer's bias/weights.
  - PSUM banks (8 total): pa(1) samp_pre per pair; zt(1) conv_pre+d per
    group of 2 pairs; pc(4) h1_pre per group; pd(1) h2_pre per pair;
    pe(1) window accumulator: the E matmuls of the 4 pairs of a window
    band-stack via PSUM accumulation, then one DVE add (+eb3) drains to
    SBUF and the output DMA ships it (DMA cannot read PSUM).
  - Emission is software-pipelined with explicit per-engine slot orders so
    the ScalarE (bottleneck engine ~79us busy) never waits:
      iter i: Act: tanhD(i-2,1) tanhA(i+1,p0) tanhA(i+1,p1) sig(i+1)
                   tanhC(i) tanhD(i-1,0)
              PE:  C(i)x8  E(i-2,1) A(i+1)x4 B(i+1)x2 D(i-1,0)x2
                   E(i-1,0) D(i-1,1)x2
"""
import numpy as np
from contextlib import ExitStack

B_TOTAL = 4194304
N_CORES = 8
R_CORE = B_TOTAL // N_CORES        # 524288
G = 16
COLS = 512
ST_ROWS = G * COLS                  # 8192
N_ST = R_CORE // ST_ROWS            # 64
N_PAIRS = N_ST // 2                 # 32 pairs (16384 rows each)
N_G = N_PAIRS // 2                  # 16 groups (2 pairs)
N_W = N_PAIRS // 4                  # 8 windows (4 pairs)

MAT_NAMES = ["A_samp", "A_conv_e", "A_conv_o", "B_e", "B_o",
             "C1_lo", "C1_hi", "C2_lo_e", "C2_lo_o", "C2_hi_e", "C2_hi_o",
             "D_lo", "D_hi", "E_0", "E_1", "E_2", "E_3"]
BIAS_NAMES = ["tanhA", "sigZ", "tanhC", "tanhD", "eb3"]


# ---------------- host-side weight/layout construction ----------------

def _band_block(W, n_in, n_out, col0=0, row0=0, mirror=True):
    """lhsT with rows row0+16f+g -> cols col0+16o+g weighted W[f,o].
    mirror=True replicates the [0,64) block into [64,128) (lo|hi halves)."""
    L = np.zeros((128, 128), np.float32)
    W = np.asarray(W, np.float32).reshape(n_in, n_out)
    for f in range(n_in):
        for o in range(n_out):
            w = W[f, o]
            if w == 0.0:
                continue
            for g in range(G):
                L[row0 + 16 * f + g, col0 + 16 * o + g] = w
    if mirror:
        L[64:, 64:] = L[:64, :64]
        L[64:, :64] = 0.0
    return L


def _build_weights(conv_w, conv_b, sW1, sb1, sW2, sb2,
                   eW1, eb1, eW2, eb2, eW3, eb3):
    mats = {}
    Wsamp = np.zeros((4, 4), np.float32)
    Wsamp[0:2, :] = sW1
    mats["A_samp"] = _band_block(Wsamp, 4, 4)
    cw = np.asarray(conv_w, np.float32).reshape(4, 1)
    mats["A_conv_e"] = _band_block(cw, 4, 1, col0=0)
    mats["A_conv_o"] = _band_block(cw, 4, 1, col0=32)
    dw = (sW2[:, 0] - sW2[:, 1]).reshape(4, 1)
    mats["B_e"] = _band_block(dw, 4, 1, col0=16)
    mats["B_o"] = _band_block(dw, 4, 1, col0=48)
    # C banks are per-supertile: bankCA = all 8 h1 bands of the LO supertile
    # (128 partitions), bankCB = HI supertile. One tanh bias vector for both.
    mats["C1_lo"] = _band_block(eW1[0:4, :], 4, 8, mirror=False)
    mats["C1_hi"] = _band_block(eW1[0:4, :], 4, 8, row0=64, mirror=False)
    W2 = np.stack([eW1[4], eW1[5] - eW1[6]], axis=0)  # (2,8)
    mats["C2_lo_e"] = _band_block(W2, 2, 8, row0=0, mirror=False)
    mats["C2_lo_o"] = _band_block(W2, 2, 8, row0=32, mirror=False)
    mats["C2_hi_e"] = _band_block(W2, 2, 8, row0=64, mirror=False)
    mats["C2_hi_o"] = _band_block(W2, 2, 8, row0=96, mirror=False)
    # D: contraction over all 8 h1 bands (128 partitions of one ST's bank);
    # h2_lo lands in cols [0,64), h2_hi in cols [64,128) of bankD.
    mats["D_lo"] = _band_block(eW2, 8, 4, mirror=False)
    mats["D_hi"] = _band_block(eW2, 8, 4, col0=64, mirror=False)
    for k in range(4):
        mats[f"E_{k}"] = _band_block(eW3.reshape(4, 1), 4, 1, col0=16 * k)

    def rep_half(vals4):
        v = np.repeat(np.asarray(vals4, np.float32), G)
        return np.concatenate([v, v])

    biases = {}
    biases["tanhA"] = rep_half(sb1)
    d_bias = np.float32(sb2[0] - sb2[1])
    biases["sigZ"] = rep_half([conv_b[0], d_bias, conv_b[0], d_bias])
    eb1_eff = (eb1 + eW1[6]).astype(np.float32)
    biases["tanhC"] = np.repeat(eb1_eff, G)  # 8 bands x 16 = 128
    biases["tanhD"] = rep_half(eb2)
    biases["eb3"] = np.full(128, np.float32(eb3[0]))

    Wpack = np.stack([mats[n] for n in MAT_NAMES], axis=1)  # (128, n_mats, 128)
    Wpack = np.ascontiguousarray(Wpack.reshape(128, len(MAT_NAMES) * 128))
    Bpack = np.ascontiguousarray(
        np.stack([biases[n] for n in BIAS_NAMES], axis=1))  # (128, 5)
    return Wpack, Bpack


def _pack_inputs(x):
    """x: (R_CORE, 4) -> (N_PAIRS, 128, 512) float32."""
    xv = x.reshape(N_W, 8, G, COLS, 4)       # [w, stidx, g, c, f]
    xv = xv.reshape(N_W, 2, 4, G, COLS, 4)   # [w, half, k, g, c, f]
    xv = xv.transpose(0, 2, 1, 5, 3, 4)      # [w, k, half, f, g, c]
    return np.ascontiguousarray(xv.reshape(N_PAIRS, 128, COLS), np.float32)


# ---------------- device program ----------------

_CACHED = {}


def _build_program():
    import concourse.bacc as bacc
    import concourse.tile as tile
    from concourse import mybir

    F32 = mybir.dt.float32
    F32R = mybir.dt.float32r
    AF = mybir.ActivationFunctionType

    nc = bacc.Bacc("TRN2", target_bir_lowering=False, debug=False)
    x_d = nc.dram_tensor("X", [N_PAIRS, 128, COLS], F32R, kind="ExternalInput")
    w_d = nc.dram_tensor("W", [128, len(MAT_NAMES) * 128], F32R,
                         kind="ExternalInput")
    b_d = nc.dram_tensor("BIAS", [128, len(BIAS_NAMES)], F32,
                         kind="ExternalInput")
    y_d = nc.dram_tensor("Y", [N_W, 128, COLS], F32, kind="ExternalOutput")

    M = {n: i for i, n in enumerate(MAT_NAMES)}
    BI = {n: i for i, n in enumerate(BIAS_NAMES)}

    with tile.TileContext(nc) as tc, ExitStack() as ctx:
        const = ctx.enter_context(tc.tile_pool(name="const", bufs=1))
        xp = ctx.enter_context(tc.tile_pool(name="xp", bufs=10))
        sampp = ctx.enter_context(tc.tile_pool(name="sampp", bufs=4))
        w4p = ctx.enter_context(tc.tile_pool(name="w4p", bufs=2))
        h1p = ctx.enter_context(tc.tile_pool(name="h1p", bufs=3))
        h2p = ctx.enter_context(tc.tile_pool(name="h2p", bufs=3))
        accp = ctx.enter_context(tc.tile_pool(name="accp", bufs=2))
        pA = ctx.enter_context(tc.tile_pool(name="pA", bufs=1, space="PSUM"))
        pZ = ctx.enter_context(tc.tile_pool(name="pZ", bufs=1, space="PSUM"))
        pC = ctx.enter_context(tc.tile_pool(name="pC", bufs=1, space="PSUM"))
        pD = ctx.enter_context(tc.tile_pool(name="pD", bufs=1, space="PSUM"))
        pE = ctx.enter_context(tc.tile_pool(name="pE", bufs=1, space="PSUM"))

        # --- prologue DMAs: bias first (tiny -> unblocks ACT warm-up),
        # then weights split into chunks interleaved with the first x tiles
        # so no single transfer stalls the pipeline head.
        bt = const.tile([128, len(BIAS_NAMES)], F32)
        # Warm the ACT tables at t=0. Sigmoid FIRST: the table-load pass then
        # picks the set that contains both sigmoid and tanh, so only one
        # 1.3us table load is ever paid (tanh-first would load a tanh-only
        # set and reload for sigmoid).
        warm = const.tile([128, 1], F32)
        nc.scalar.activation(warm[:], bt[:, 0:1], AF.Sigmoid)
        nc.scalar.activation(warm[:], bt[:, 0:1], AF.Tanh)

        wt_a = const.tile([128, 5 * 128], F32R)       # A_samp..B_o
        wt_c1 = const.tile([128, 4 * 128], F32R)      # C1_lo..C2_lo_o
        wt_c2 = const.tile([128, 4 * 128], F32R)      # C2_hi_e..D_hi
        wt_e = const.tile([128, 4 * 128], F32R)       # E_0..E_3

        # PE p-state pre-warm: dummy matmuls on a DMA-filled scratch tile keep
        # the PE busy from t~2.2 so the cost model's 3us ramp completes before
        # the first real matmul (which would otherwise run at 2-4x cycle).
        pewarm = const.tile([128, 256], F32R)
        nc.sync.dma_start(out=pewarm[:], in_=w_d[:, 0:256])
        warm_ps = pE.tile([128, COLS], F32, tag="pe")
        for _ in range(8):
            nc.tensor.matmul(warm_ps[:, 0:256], pewarm[:, 0:128], pewarm[:],
                             start=True, stop=True, skip_group_check=True)

        xt = {}  # pair index -> x tile

        def dma_x(g):
            for kk in (0, 1):
                p = 2 * g + kk
                t = xp.tile([128, COLS], F32R, tag="x2")
                nc.sync.dma_start(out=t[:], in_=x_d[p])
                xt[p] = t

        dma_x(0)
        nc.sync.dma_start(out=wt_a[:, 0:128], in_=w_d[:, 0:128])
        nc.sync.dma_start(out=bt[:], in_=b_d[:])
        nc.sync.dma_start(out=wt_a[:, 128:640], in_=w_d[:, 128:640])
        dma_x(1)
        nc.sync.dma_start(out=wt_c1[:], in_=w_d[:, 640:1152])
        dma_x(2)
        nc.sync.dma_start(out=wt_c2[:], in_=w_d[:, 1152:1664])
        nc.sync.dma_start(out=wt_e[:], in_=w_d[:, 1664:2176])

        def W(name):
            m = M[name]
            if m < 5:
                return wt_a[:, m * 128:(m + 1) * 128]
            if m < 9:
                return wt_c1[:, (m - 5) * 128:(m - 4) * 128]
            if m < 13:
                return wt_c2[:, (m - 9) * 128:(m - 8) * 128]
            return wt_e[:, (m - 13) * 128:(m - 12) * 128]

        def bias(name):
            return bt[:, BI[name]:BI[name] + 1]

        # --- pipeline state
        samp = {}   # pair -> samp tile (tanhA out)
        w4 = {}     # group -> sigmoid out tile
        h1 = {}     # group -> tanhC out tile (128, 2048)
        h2 = {}     # (group, kk) -> tanhD out tile
        pa_t = {}   # pair -> pa PSUM tile
        zt_t = {}   # group -> zt PSUM tile
        pc_t = {}   # group -> pc PSUM tile
        pd_t = {}   # (group, kk) -> pd PSUM tile
        pe_cur = [None]  # current window accumulator

        def emit_Asamp(g, kk):
            p = 2 * g + kk
            pa = pA.tile([128, COLS], F32, tag="pa")
            pa_t[p] = pa
            nc.tensor.matmul(pa[:], W("A_samp"), xt[p][:],
                             start=True, stop=True, skip_group_check=True)

        def emit_Aconv(g, kk):
            p = 2 * g + kk
            eo = "e" if kk == 0 else "o"
            if kk == 0:
                zt = pZ.tile([128, COLS], F32, tag="pz")
                zt_t[g] = zt
            nc.tensor.matmul(zt_t[g][:], W(f"A_conv_{eo}"), xt[p][:],
                             start=(kk == 0), stop=False,
                             skip_group_check=True)

        def emit_tanhA(g, kk):
            p = 2 * g + kk
            s = sampp.tile([128, COLS], F32R, tag="samp")
            samp[p] = s
            nc.scalar.activation(s[:], pa_t[p][:], AF.Tanh, bias=bias("tanhA"))
            del pa_t[p]

        def emit_B(g, kk):
            eo = "e" if kk == 0 else "o"
            nc.tensor.matmul(zt_t[g][:], W(f"B_{eo}"), samp[2 * g + kk][:],
                             start=False, stop=(kk == 1),
                             skip_group_check=True)

        def emit_sig(g):
            t = w4p.tile([128, COLS], F32R, tag="w4")
            w4[g] = t
            nc.scalar.activation(t[:], zt_t[g][:], AF.Sigmoid,
                                 bias=bias("sigZ"))
            del zt_t[g]

        def emit_C1(g, kk, half, alloc=False, del_x=False):
            if alloc:
                pc = pC.tile([128, 4 * COLS], F32, tag="pc")
                pc_t[g] = pc
            p = 2 * g + kk
            q = kk * 2 * COLS + (0 if half == "lo" else COLS)
            nc.tensor.matmul(pc_t[g][:, q:q + COLS], W(f"C1_{half}"),
                             xt[p][:], start=True, stop=False,
                             skip_group_check=True)
            if del_x:
                del xt[p]

        def emit_C2(g, kk, half):
            eo = "e" if kk == 0 else "o"
            q = kk * 2 * COLS + (0 if half == "lo" else COLS)
            nc.tensor.matmul(pc_t[g][:, q:q + COLS], W(f"C2_{half}_{eo}"),
                             w4[g][:], start=False, stop=True,
                             skip_group_check=True)

        def emit_tanhC(g):
            t = h1p.tile([128, 4 * COLS], F32R, tag="h1")
            h1[g] = t
            nc.scalar.activation(t[:], pc_t[g][:], AF.Tanh, bias=bias("tanhC"))
            del pc_t[g]
            if g >= 1:
                del w4[g - 1]

        def emit_D(g, kk, pool=None):
            q = kk * 2 * COLS
            if pool is None:
                pd = pD.tile([128, COLS], F32, tag="pd")
            else:
                pd = pool.tile([128, COLS], F32, tag="pa")
            pd_t[(g, kk)] = pd
            nc.tensor.matmul(pd[:], W("D_lo"), h1[g][:, q:q + COLS],
                             start=True, stop=False, skip_group_check=True)
            nc.tensor.matmul(pd[:], W("D_hi"), h1[g][:, q + COLS:q + 2 * COLS],
                             start=False, stop=True, skip_group_check=True)

        def emit_tanhD(g, kk):
            t = h2p.tile([128, COLS], F32R, tag="h2")
            h2[(g, kk)] = t
            nc.scalar.activation(t[:], pd_t[(g, kk)][:], AF.Tanh,
                                 bias=bias("tanhD"))
            del pd_t[(g, kk)]

        def emit_E(g, kk):
            kg = (2 * g + kk) % 4          # window-local pair index
            if kg == 0:
                pe = pE.tile([128, COLS], F32, tag="pe")
                pe_cur[0] = pe
            nc.tensor.matmul(pe_cur[0][:], W(f"E_{kg}"), h2[(g, kk)][:],
                             start=(kg == 0), stop=(kg == 3),
                             skip_group_check=True)
            del h2[(g, kk)]

        def emit_out(w):
            acc = accp.tile([128, COLS], F32, tag="acc")
            nc.vector.tensor_scalar_add(acc[:], pe_cur[0][:], bias("eb3"))
            nc.sync.dma_start(out=y_d[w], in_=acc[:])

        # --- software-pipelined main loop (cyclic schedule, period ~4.96us,
        # ScalarE-bound and gapless in steady state).
        # Act queue/period p: tanhD(p-3,0) tanhA(p,p0) tanhD(p-3,1)
        #                     tanhA(p,p1) tanhC(p-1) sig(p)
        # PE queue/period p:  Aconv(p,0) C1hi(p-1,1) C2loE(p-1) D(p-3,1)
        #                     A(p,1) C2hiE(p-1) C2loO(p-1) C2hiO(p-1)
        #                     B_e(p) B_o(p) D(p-2,0) E(p-3,*) Asamp(p+1,0)
        #                     C1(p)x3
        # The D stage lags 3 groups so its matmuls run during tanhC; C groups
        # split across the period boundary (C1 tail needs only x; C2 needs
        # the sigmoid output of the same period).
        emit_Asamp(0, 0)
        for p in range(0, N_G + 1):
            if 3 <= p + 3 < N_G:
                dma_x(p + 3)
            if 0 <= p - 3 < N_G:
                emit_tanhD(p - 3, 0)
            if 0 <= p < N_G:
                emit_Aconv(p, 0)
            if 0 <= p - 1 < N_G:
                emit_C1(p - 1, 1, "hi", del_x=True)
            if 0 <= p < N_G:
                emit_tanhA(p, 0)
            if 0 <= p - 1 < N_G:
                emit_C2(p - 1, 0, "lo")
            if 0 <= p - 3 < N_G:
                emit_D(p - 3, 1)
                emit_tanhD(p - 3, 1)
            if 0 <= p < N_G:
                emit_Asamp(p, 1)
                emit_Aconv(p, 1)
                emit_tanhA(p, 1)
            if 0 <= p - 1 < N_G:
                emit_C2(p - 1, 0, "hi")
                emit_C2(p - 1, 1, "lo")
                emit_C2(p - 1, 1, "hi")
                emit_tanhC(p - 1)
            if 0 <= p < N_G:
                emit_B(p, 0)
                emit_B(p, 1)
            if 0 <= p - 2 < N_G:
                emit_D(p - 2, 0)
            if 0 <= p - 3 < N_G:
                emit_E(p - 3, 0)
                emit_E(p - 3, 1)
                if (p - 3) % 2 == 1:
                    emit_out((p - 3) // 2)
            if 0 <= p < N_G:
                emit_sig(p)
            if 0 <= p + 1 < N_G:
                emit_Asamp(p + 1, 0)
            if 0 <= p < N_G:
                emit_C1(p, 0, "lo", alloc=True)
                emit_C1(p, 0, "hi", del_x=True)
                emit_C1(p, 1, "lo")

        # --- eager drain of the last two groups: alternate D tiles between
        # the pD bank and the now-idle pA bank so the final four tanhDs run
        # back-to-back on ScalarE instead of serializing through one bank.
        gl = N_G - 2
        emit_D(gl, 1, pool=pA)
        emit_tanhD(gl, 0)
        emit_E(gl, 0)
        emit_tanhD(gl, 1)
        emit_D(gl + 1, 0)
        emit_E(gl, 1)
        emit_tanhD(gl + 1, 0)
        emit_D(gl + 1, 1, pool=pA)
        emit_E(gl + 1, 0)
        emit_tanhD(gl + 1, 1)
        emit_E(gl + 1, 1)
        emit_out(N_W - 1)

    nc.compile()
    return nc


def kernel(**inputs):
    from concourse.bass_utils import run_bass_kernel_spmd

    inputs = {k: np.asarray(v, np.float32) for k, v in inputs.items()}
    x = inputs["inputs"]
    Wpack, Bpack = _build_weights(
        inputs["conv_w"], inputs["conv_b"], inputs["sW1"], inputs["sb1"],
        inputs["sW2"], inputs["sb2"], inputs["eW1"], inputs["eb1"],
        inputs["eW2"], inputs["eb2"], inputs["eW3"], inputs["eb3"])

    if "nc" not in _CACHED:
        _CACHED["nc"] = _build_program()
    nc = _CACHED["nc"]

    in_maps = []
    for c in range(N_CORES):
        xc = x[c * R_CORE:(c + 1) * R_CORE]
        in_maps.append({"X": _pack_inputs(xc), "W": Wpack, "BIAS": Bpack})

    res = run_bass_kernel_spmd(nc, in_maps, list(range(N_CORES)))
    out = np.concatenate(
        [res.results[c]["Y"].reshape(R_CORE, 1) for c in range(N_CORES)],
        axis=0)
    return out.astype(np.float32)



# revision 17
# speedup vs baseline: 1.0211x; 1.0211x over previous
"""Trainium2 Bass kernel for nn_EstimatorQNNGen104 (dense tiny-MLP over 4.2M rows).

Pure data parallel over 8 NeuronCores (R_core = 524288 rows/core), bf16 data +
fp32r weights.  Per core the batch is processed in 16 "groups" of 2 pairs
(4 supertiles of 8192 rows; banding: partition = 16*feature + rowgroup,
512 cols per band).

Key structure (vs a naive port of the reference):
  - softmax(2) -> sigmoid(d), and every sigmoid is computed as
    0.5 + 0.5*tanh(v/2) with the affine part folded into the next layer's
    weights/biases, so the only activation function used anywhere is tanh.
  - The estimator's first layer consumes a per-ST "combined tile" xc
    [96, 512] = [x (64p) | tanh(c/2) (16p) | tanh(d/2) (16p)] so the whole
    7->8 layer is ONE matmul per supertile (the sampler/conv features are
    copied into xc by cheap DVE tensor_scalar copies).
  - The sampler hidden tanh (4 units) is offloaded off the ScalarE:
    Pool drains PSUM with (+bias, min 1), a single custom DVE instruction
    (deg-7 odd polynomial with one-sided clamp, 8 ALU stages) finishes
    tanh(3.2*u).  Scales 1/3.2 and the poly's leading coefficient are folded
    into the A / B matmul weights.  ScalarE keeps the accuracy-critical
    tanh's (zt, h1, h2) exactly.
  - E-stage outputs of a group's 2 pairs land in 64 partitions of the pa
    PSUM bank (reused late in the period), drained by one DVE add(eb3).
  - PSUM: pa/pe 1 bank, zt 1, pd 2, pc 4 = 8 banks exactly.
"""
import numpy as np
from contextlib import ExitStack

import ml_dtypes

B_TOTAL = 4194304
N_CORES = 8
R_CORE = B_TOTAL // N_CORES        # 524288
G = 16
COLS = 512
ST_ROWS = G * COLS                  # 8192
N_ST = R_CORE // ST_ROWS            # 64
N_PAIRS = N_ST // 2                 # 32
N_G = N_PAIRS // 2                  # 16 groups (2 pairs, 4 STs)

CLAMP = 3.2                         # tanh(CLAMP)=0.9967; poly fitted on [-1,1]

BIAS_NAMES = ["ztb", "eb2b", "eb1b", "sab", "eb3b"]


# ---------------- poly fit (deg-7 odd, approx-minimax) ----------------

def _fit_tanh7(C):
    u = np.linspace(0, 1, 20001)
    y = np.tanh(C * u)
    A = np.stack([u ** (2 * k + 1) for k in range(4)], axis=1)
    w = np.ones_like(u)
    coef = None
    for _ in range(80):
        coef, *_ = np.linalg.lstsq(A * w[:, None], y * w, rcond=None)
        r = np.abs(A @ coef - y)
        w *= (1.0 + r / (r.max() + 1e-12)) ** 2
        w /= w.max()
    q = np.polynomial.Polynomial(coef)
    roots = q.roots()
    rr = [x for x in roots if abs(x.imag) < 1e-9]
    cc = [x for x in roots if x.imag > 1e-9]
    assert len(rr) == 1 and len(cc) == 1, roots
    r1 = float(rr[0].real)
    p1 = float(-2 * cc[0].real)
    q1 = float(abs(cc[0]) ** 2)
    k = float(coef[-1])
    return k, r1, p1, q1

POLY_K, POLY_R1, POLY_P1, POLY_Q1 = _fit_tanh7(CLAMP)


def _poly_ref(in0, in1, c0, c1, c2):
    # matches the Spec body: v = min(in0, 1); (t-c0)*((t+c1)*t+c2)*v
    v = np.minimum(np.asarray(in0, np.float32), 1.0)
    t = v * v
    return ((t - c0) * ((t + c1) * t + c2) * v).astype(np.float32)


_DVE_OP = [None]


def _get_tanh_op():
    if _DVE_OP[0] is not None:
        return _DVE_OP[0]
    from concourse.dve_spec import (
        Spec, Src0, C0, C1, C2, One, minn, sq, lower, _has_src1,
    )
    from concourse.dve_uop import DveOpSpec
    from concourse.dve_ops import DveOp, OPS, CUSTOM_DVE_SPECS, _SUB_OPCODE_FOR_NAME

    name = "TANH7_ANT_EQNN"
    if name not in _SUB_OPCODE_FOR_NAME:
        v = minn(Src0, One)
        t = sq(v)
        spec = Spec(body=((t - C0) * ((t + C1) * t + C2)) * v,
                    reference=_poly_ref)
        row = max(_SUB_OPCODE_FOR_NAME.values()) + 1
        assert row < 0x20
        _SUB_OPCODE_FOR_NAME[name] = row
        shas = {}
        for ver in ("v3", "v4"):
            s = DveOpSpec(name=name, opcode=row, uops=lower(spec, ver=ver),
                          rd1_en=_has_src1(spec))
            shas[ver] = s.sha(ver)
        op = DveOp(name, spec, subdim=False, uops_sha=shas)
        OPS.append(op)
        CUSTOM_DVE_SPECS[name] = spec
        _DVE_OP[0] = op
    else:
        from concourse.dve_ops import OPS as _ops
        _DVE_OP[0] = next(o for o in _ops if o.name == name)
    return _DVE_OP[0]


# ---------------- host-side weights ----------------

def _build_weights(conv_w, conv_b, sW1, sb1, sW2, sb2,
                   eW1, eb1, eW2, eb2, eW3, eb3):
    """All lhsT matrices [128 or 96, 128] fp32-encoded (fed as float32r)."""
    f64 = np.float64
    conv_w = np.asarray(conv_w, f64).reshape(4)
    sW1, sb1 = np.asarray(sW1, f64), np.asarray(sb1, f64)
    sW2, sb2 = np.asarray(sW2, f64), np.asarray(sb2, f64)
    eW1, eb1 = np.asarray(eW1, f64), np.asarray(eb1, f64)
    eW2, eb2 = np.asarray(eW2, f64), np.asarray(eb2, f64)
    eW3, eb3 = np.asarray(eW3, f64), np.asarray(eb3, f64)

    mats = {}

    # --- A (sampler pre-act / CLAMP): x pair tile -> pa bank.
    # pair tile partitions: 64*half + 16*f + g ; out: 64*half + 16*u + g.
    A = np.zeros((128, 128), f64)
    for h in (0, 1):
        for f in range(2):
            for u in range(4):
                wv = sW1[f, u] / CLAMP
                if wv == 0.0:
                    continue
                for g in range(G):
                    A[64 * h + 16 * f + g, 64 * h + 16 * u + g] = wv
    mats["A0"] = A          # same lhsT for both pairs
    mats["A1"] = A

    # --- CONV (pair kk): x -> zt bands; value c/2 (+bias via ACT bias).
    # zt partitions: 64*kk + 32*h + {0-15: tc, 16-31: td}.
    for kk in (0, 1):
        M = np.zeros((128, 128), f64)
        for h in (0, 1):
            for f in range(4):
                for g in range(G):
                    M[64 * h + 16 * f + g, 64 * kk + 32 * h + 0 + g] = \
                        conv_w[f] * 0.5
        mats[f"CONV{kk}"] = M
    # --- B (pair kk): samp -> zt d-band; d = dw^T tanh + db; samp holds
    # tanh/POLY_K so scale by POLY_K; also *0.5 for the sigma->tanh trick.
    dw = (sW2[:, 0] - sW2[:, 1])
    for kk in (0, 1):
        M = np.zeros((128, 128), f64)
        for h in (0, 1):
            for u in range(4):
                wv = dw[u] * POLY_K * 0.5
                for g in range(G):
                    M[64 * h + 16 * u + g, 64 * kk + 32 * h + 16 + g] = wv
        mats[f"B{kk}"] = M
    # --- C: combined tile [96,512] -> h1 pre (8 out bands, 128 partitions).
    # xc partitions: 0-63: 16f+g (x); 64-79: tc; 80-95: td.
    w_c = eW1[4]
    w_s = eW1[5] - eW1[6]
    C = np.zeros((96, 128), f64)
    for f in range(4):
        for o in range(8):
            wv = eW1[f, o]
            for g in range(G):
                C[16 * f + g, 16 * o + g] = wv
    for o in range(8):
        for g in range(G):
            C[64 + g, 16 * o + g] = 0.5 * w_c[o]
            C[80 + g, 16 * o + g] = 0.5 * w_s[o]
    mats["C"] = C

    # --- D (contract one ST's h1 [8 bands] -> h2 [4 bands]).
    # pd layout per pair: cols [0,512): lo-ST h2 at partitions 0-63,
    # hi-ST h2 at 64-127  -> two lhsT: D_lo (cols 0-63), D_hi (cols 64-127).
    for nm, col0 in (("D_lo", 0), ("D_hi", 64)):
        M = np.zeros((128, 128), f64)
        for f in range(8):
            for o in range(4):
                wv = eW2[f, o]
                for g in range(G):
                    M[16 * f + g, col0 + 16 * o + g] = wv
        mats[nm] = M

    # --- E (pair kk): h2 [128] -> out band; out partitions 32*kk+16*h+g.
    for kk in (0, 1):
        M = np.zeros((128, 128), f64)
        for h in (0, 1):
            for o in range(4):
                wv = eW3[o, 0]
                for g in range(G):
                    M[64 * h + 16 * o + g, 32 * kk + 16 * h + g] = wv
        mats[f"E{kk}"] = M

    # --- biases (fp32, [128,1] each) ---
    biases = {}
    ztb = np.zeros(128, f64)
    for kk in (0, 1):
        for h in (0, 1):
            ztb[64 * kk + 32 * h + 0:64 * kk + 32 * h + 16] = conv_b[0] * 0.5
            ztb[64 * kk + 32 * h + 16:64 * kk + 32 * h + 32] = \
                (sb2[0] - sb2[1]) * 0.5
    biases["ztb"] = ztb
    eb2b = np.zeros(128, f64)
    for h in (0, 1):
        for o in range(4):
            eb2b[64 * h + 16 * o:64 * h + 16 * o + 16] = eb2[o]
    biases["eb2b"] = eb2b
    eb1_eff = eb1 + eW1[6] + 0.5 * w_c + 0.5 * w_s
    eb1b = np.zeros(128, f64)
    for o in range(8):
        eb1b[16 * o:16 * o + 16] = eb1_eff[o]
    biases["eb1b"] = eb1b
    sab = np.zeros(128, f64)
    for h in (0, 1):
        for u in range(4):
            sab[64 * h + 16 * u:64 * h + 16 * u + 16] = sb1[u] / CLAMP
    biases["sab"] = sab
    biases["eb3b"] = np.full(128, eb3[0], f64)

    names = ["A0", "A1", "CONV0", "CONV1", "B0", "B1", "D_lo", "D_hi",
             "E0", "E1"]
    b16 = ml_dtypes.bfloat16
    Wpack = np.stack([mats[n] for n in names], axis=1)       # (128, 10, 128)
    Wpack = np.ascontiguousarray(
        Wpack.reshape(128, len(names) * 128).astype(b16))
    Cpack = np.ascontiguousarray(mats["C"].astype(b16))      # (96,128)
    Bpack = np.ascontiguousarray(np.stack(
        [biases[n] for n in BIAS_NAMES], axis=1).astype(np.float32))
    return Wpack, Cpack, Bpack, names


def _pack_inputs(x):
    """x (R_CORE,4) fp32 -> (pair tiles, xc tiles) in bf16.

    pair tile p: [128, 512], partition 64*h + 16*f + g, col c
                 = x[ST(2p+h) row g*512+c, f]
    xc tile g:   [64, 2048], partition 16*f + gg, col 512*s + c
                 = x[ST(4g+s) row gg*512+c, f]   (s = 2*kk + h)
    """
    xs = x.reshape(N_ST, G, COLS, 4)                  # [st, g, c, f]
    stf = np.ascontiguousarray(xs.transpose(0, 3, 1, 2))  # [st, f, g, c]
    stf = stf.reshape(N_ST, 64, COLS)                 # partition 16f+g
    pair = stf.reshape(N_PAIRS, 2, 64, COLS).reshape(N_PAIRS, 128, COLS)
    xc = stf.reshape(N_G, 4, 64, COLS).transpose(0, 2, 1, 3)
    xc = np.ascontiguousarray(xc.reshape(N_G, 64, 4 * COLS))
    b16 = ml_dtypes.bfloat16
    return (np.ascontiguousarray(pair).astype(b16),
            xc.astype(b16))


def _unpack_out(y):
    """y (N_G, 64, 512) -> (R_CORE, 1) fp32.
    partition 32*kk + 16*h + g, col c -> ST(4g'+2kk+h) row g*512+c."""
    y = np.asarray(y, np.float32).reshape(N_G, 2, 2, G, COLS)  # [g,kk,h,gg,c]
    y = y.transpose(0, 1, 2, 3, 4)     # already [g, kk, h, gg, c]
    # ST index = 4g + 2kk + h ; rows = st*8192 + gg*512 + c
    return np.ascontiguousarray(y.reshape(R_CORE, 1))


# ---------------- device program ----------------

_CACHED = {}


def _build_program():
    import concourse.bacc as bacc
    import concourse.tile as tile
    from concourse import mybir

    F32 = mybir.dt.float32
    F32R = mybir.dt.float32r
    BF16 = mybir.dt.bfloat16
    AF = mybir.ActivationFunctionType
    ALU = mybir.AluOpType
    tanh_op = _get_tanh_op()

    nc = bacc.Bacc("TRN2", target_bir_lowering=False, debug=False)
    x_d = nc.dram_tensor("X", [N_PAIRS, 128, COLS], BF16, kind="ExternalInput")
    xc_d = nc.dram_tensor("XC", [N_G, 64, 4 * COLS], BF16,
                          kind="ExternalInput")
    w_d = nc.dram_tensor("W", [128, 10 * 128], BF16, kind="ExternalInput")
    wc_d = nc.dram_tensor("WC", [96, 128], BF16, kind="ExternalInput")
    b_d = nc.dram_tensor("BIAS", [128, len(BIAS_NAMES)], F32,
                         kind="ExternalInput")
    y_d = nc.dram_tensor("Y", [N_G, 64, COLS], BF16, kind="ExternalOutput")

    WN = {n: i for i, n in enumerate(
        ["A0", "A1", "CONV0", "CONV1", "B0", "B1", "D_lo", "D_hi",
         "E0", "E1"])}
    BI = {n: i for i, n in enumerate(BIAS_NAMES)}

    with tile.TileContext(nc) as tc, ExitStack() as ctx:
        const = ctx.enter_context(tc.tile_pool(name="const", bufs=1))
        xp = ctx.enter_context(tc.tile_pool(name="xp", bufs=8))
        xcp = ctx.enter_context(tc.tile_pool(name="xcp", bufs=4))
        sap = ctx.enter_context(tc.tile_pool(name="sap", bufs=2))
        sampp = ctx.enter_context(tc.tile_pool(name="sampp", bufs=2))
        ztaup = ctx.enter_context(tc.tile_pool(name="ztaup", bufs=2))
        h1p = ctx.enter_context(tc.tile_pool(name="h1p", bufs=2))
        h2p = ctx.enter_context(tc.tile_pool(name="h2p", bufs=2))
        accp = ctx.enter_context(tc.tile_pool(name="accp", bufs=2))
        # PSUM: pa(1, shared with pe) + zt(1) + pd(2) + pc(4) = 8 banks
        pA = ctx.enter_context(tc.tile_pool(name="pA", bufs=1, space="PSUM"))
        pZ = ctx.enter_context(tc.tile_pool(name="pZ", bufs=1, space="PSUM"))
        pD = ctx.enter_context(tc.tile_pool(name="pD", bufs=1, space="PSUM"))
        pC = ctx.enter_context(tc.tile_pool(name="pC", bufs=1, space="PSUM"))

        bt = const.tile([128, len(BIAS_NAMES)], F32)
        # Warm the ACT table (tanh only) immediately.
        warm = const.tile([128, 1], F32)
        nc.scalar.activation(warm[:], bt[:, 0:1], AF.Tanh)

        wt = const.tile([128, 10 * 128], BF16)
        wct = const.tile([96, 128], BF16)

        # PE p-state pre-warm (cost model: 3us ramp to full speed).
        pewarm = const.tile([128, 256], BF16)
        nc.sync.dma_start(out=pewarm[:], in_=w_d[:, 0:256])
        warm_ps = pC.tile([128, 2048], F32, tag="pc")
        for _ in range(8):
            nc.tensor.matmul(warm_ps[:, 0:256], pewarm[:, 0:128], pewarm[:],
                             start=True, stop=True, skip_group_check=True)

        xt = {}     # pair -> x tile
        xct = {}    # group -> xc tile

        def dma_x(g):
            for kk in (0, 1):
                p = 2 * g + kk
                t = xp.tile([128, COLS], BF16, tag="x2")
                nc.sync.dma_start(out=t[:], in_=x_d[p])
                xt[p] = t
            t = xcp.tile([96, 4 * COLS], BF16, tag="xc")
            nc.sync.dma_start(out=t[0:64, :], in_=xc_d[g])
            xct[g] = t

        dma_x(0)
        nc.sync.dma_start(out=wt[:, 0:256], in_=w_d[:, 0:256])
        nc.sync.dma_start(out=bt[:], in_=b_d[:])
        nc.sync.dma_start(out=wt[:, 256:640], in_=w_d[:, 256:640])
        dma_x(1)
        nc.sync.dma_start(out=wct[:], in_=wc_d[:])
        nc.sync.dma_start(out=wt[:, 640:1280], in_=w_d[:, 640:1280])
        dma_x(2)

        def W(name):
            m = WN[name]
            return wt[:, m * 128:(m + 1) * 128]

        def bias(name):
            return bt[:, BI[name]:BI[name] + 1]

        pa_t = {}   # pair -> pa psum tile
        sa_t = {}   # pair -> clamped sbuf tile
        samp = {}   # pair -> poly out (tanh/POLY_K)
        zt_t = {}   # group -> zt psum
        ztau = {}   # group -> tanh(zt) sbuf  [tc|td bands]
        pc_t = {}   # group -> pc psum [128, 2048]
        pd_t = {}   # group -> pd psum [128, 1024] (pair0|pair1)
        h1 = {}     # group -> h1 sbuf [128, 2048]
        h2 = {}     # group -> h2 sbuf [128, 1024]
        pe_t = {}   # group -> pe psum (pa pool, 64 partitions)

        def emit_A(g, kk):
            p = 2 * g + kk
            pa = pA.tile([128, COLS], F32, tag="pa")
            pa_t[p] = pa
            nc.tensor.matmul(pa[:], W(f"A{kk}"), xt[p][:],
                             start=True, stop=True, skip_group_check=True)

        def emit_clamp(g, kk):
            # DVE: sa = max(pa + sab, -1.0)  (upper clamp inside the DVE op;
            # GPSIMD cannot read PSUM, so this drain must be DVE)
            p = 2 * g + kk
            t = sap.tile([128, COLS], BF16, tag="sa")
            sa_t[p] = t
            nc.vector.tensor_scalar(t[:], pa_t[p][:], bias("sab"), -1.0,
                                    ALU.add, ALU.max)
            del pa_t[p]

        def emit_poly(g, kk):
            p = 2 * g + kk
            t = sampp.tile([128, COLS], BF16, tag="samp")
            samp[p] = t
            nc.vector._custom_dve(tanh_op, out=t[:], in0=sa_t[p][:],
                                  s0=POLY_R1, s1=POLY_P1, imm2=POLY_Q1)
            del sa_t[p]

        def emit_conv(g, kk):
            p = 2 * g + kk
            if kk == 0:
                zt = pZ.tile([128, COLS], F32, tag="pz")
                zt_t[g] = zt
            nc.tensor.matmul(zt_t[g][:], W(f"CONV{kk}"), xt[p][:],
                             start=(kk == 0), stop=False,
                             skip_group_check=True)
            del xt[p]

        def emit_B(g, kk):
            nc.tensor.matmul(zt_t[g][:], W(f"B{kk}"), samp[2 * g + kk][:],
                             start=False, stop=(kk == 1),
                             skip_group_check=True)
            del samp[2 * g + kk]

        def emit_tanh_zt(g):
            t = ztaup.tile([128, COLS], BF16, tag="ztau")
            ztau[g] = t
            nc.scalar.activation(t[:], zt_t[g][:], AF.Tanh, bias=bias("ztb"))
            del zt_t[g]

        def emit_copies(g):
            # 4 copies [32,512]: ztau bands -> xc partitions 64-95.
            # 3 on Pool (SBUF->SBUF is legal there), 1 on DVE for balance.
            for s in range(4):
                eng = nc.vector if s == 3 else nc.gpsimd
                eng.tensor_scalar(
                    xct[g][64:96, s * COLS:(s + 1) * COLS],
                    ztau[g][32 * s:32 * s + 32, :], 1.0, None, ALU.mult)
            del ztau[g]

        def emit_C(g, s, alloc=False):
            if alloc:
                pc = pC.tile([128, 4 * COLS], F32, tag="pc")
                pc_t[g] = pc
            nc.tensor.matmul(pc_t[g][:, s * COLS:(s + 1) * COLS],
                             wct[:], xct[g][:, s * COLS:(s + 1) * COLS],
                             start=True, stop=True, skip_group_check=True)
            if s == 3:
                del xct[g]

        def emit_tanhC(g):
            t = h1p.tile([128, 4 * COLS], BF16, tag="h1")
            h1[g] = t
            nc.scalar.activation(t[:], pc_t[g][:], AF.Tanh, bias=bias("eb1b"))
            del pc_t[g]

        def emit_D(g, kk):
            if kk == 0:
                pd = pD.tile([128, 2 * COLS], F32, tag="pd")
                pd_t[g] = pd
            q = 2 * kk * COLS
            nc.tensor.matmul(pd_t[g][:, kk * COLS:(kk + 1) * COLS],
                             W("D_lo"), h1[g][:, q:q + COLS],
                             start=True, stop=False, skip_group_check=True)
            nc.tensor.matmul(pd_t[g][:, kk * COLS:(kk + 1) * COLS],
                             W("D_hi"), h1[g][:, q + COLS:q + 2 * COLS],
                             start=False, stop=True, skip_group_check=True)
            if kk == 1:
                del h1[g]

        def emit_tanhD(g):
            t = h2p.tile([128, 2 * COLS], BF16, tag="h2")
            h2[g] = t
            nc.scalar.activation(t[:], pd_t[g][:], AF.Tanh, bias=bias("eb2b"))
            del pd_t[g]

        def emit_E(g, kk):
            if kk == 0:
                pe = pA.tile([128, COLS], F32, tag="pa")
                pe_t[g] = pe
            nc.tensor.matmul(pe_t[g][0:64, :], W(f"E{kk}")[:, 0:64],
                             h2[g][:, kk * COLS:(kk + 1) * COLS],
                             start=(kk == 0), stop=(kk == 1),
                             skip_group_check=True)
            if kk == 1:
                del h2[g]

        def emit_out(g):
            acc = accp.tile([64, COLS], BF16, tag="acc")
            nc.vector.tensor_scalar(acc[:], pe_t[g][0:64, :],
                                    bt[0:64, BI["eb3b"]:BI["eb3b"] + 1], None,
                                    ALU.add)
            nc.sync.dma_start(out=y_d[g], in_=acc[:])
            del pe_t[g]

        # ---- software pipeline ----
        # stage lags (group g emitted in period p = g + lag):
        #   A/clamp/poly/conv/B : 0   zt-tanh/copies : 1   C : 1 (late)
        #   tanhC : 2  D : 2 (late)  tanhD : 3  E : 3  out : 3 (end)
        for p in range(N_G + 4):
            if 2 <= p and p + 1 < N_G:
                dma_x(p + 1)
            g0 = p          # cohort entering
            g1 = p - 1      # zt-tanh / copies / C
            g2 = p - 2      # tanhC / D
            g3 = p - 3      # tanhD / E / out
            if 0 <= g0 < N_G:
                emit_A(g0, 0)
            if 0 <= g3 < N_G:
                emit_tanhD(g3)
            if 0 <= g1 < N_G:
                emit_tanh_zt(g1)
            if 0 <= g0 < N_G:
                emit_clamp(g0, 0)
                emit_A(g0, 1)
                emit_conv(g0, 0)
                emit_conv(g0, 1)
                emit_poly(g0, 0)
                emit_clamp(g0, 1)
            if 0 <= g2 < N_G:
                emit_tanhC(g2)
            if 0 <= g1 < N_G:
                emit_copies(g1)
            if 0 <= g2 < N_G:
                emit_D(g2, 0)
                emit_D(g2, 1)
            if 0 <= g3 < N_G:
                emit_E(g3, 0)
                emit_E(g3, 1)
            if 0 <= g1 < N_G:
                emit_C(g1, 0, alloc=True)
                emit_C(g1, 1)
                emit_C(g1, 2)
            if 0 <= g0 < N_G:
                emit_poly(g0, 1)
                emit_B(g0, 0)
                emit_B(g0, 1)
            if 0 <= g1 < N_G:
                emit_C(g1, 3)
            if 0 <= g3 < N_G:
                emit_out(g3)

    nc.compile()
    return nc


def kernel(**inputs):
    from concourse.bass_utils import run_bass_kernel_spmd

    inputs = {k: np.asarray(v, np.float32) for k, v in inputs.items()}
    x = inputs["inputs"]
    Wpack, Cpack, Bpack, _ = _build_weights(
        inputs["conv_w"], inputs["conv_b"], inputs["sW1"], inputs["sb1"],
        inputs["sW2"], inputs["sb2"], inputs["eW1"], inputs["eb1"],
        inputs["eW2"], inputs["eb2"], inputs["eW3"], inputs["eb3"])

    if "nc" not in _CACHED:
        _CACHED["nc"] = _build_program()
    nc = _CACHED["nc"]

    in_maps = []
    for c in range(N_CORES):
        xc = x[c * R_CORE:(c + 1) * R_CORE]
        pair, xcomb = _pack_inputs(xc)
        in_maps.append({"X": pair, "XC": xcomb, "W": Wpack, "WC": Cpack,
                        "BIAS": Bpack})

    res = run_bass_kernel_spmd(nc, in_maps, list(range(N_CORES)))
    out = np.concatenate(
        [_unpack_out(res.results[c]["Y"]) for c in range(N_CORES)], axis=0)
    return out.astype(np.float32)


# revision 48
# speedup vs baseline: 1.2747x; 1.2483x over previous
"""Trainium2 Bass kernel for nn_EstimatorQNNGen104 (dense tiny-MLP over 4.2M rows).

Pure data parallel over 8 NeuronCores (R_core = 524288 rows/core), bf16 data +
fp32r weights.  Per core the batch is processed in 16 "groups" of 2 pairs
(4 supertiles of 8192 rows; banding: partition = 16*feature + rowgroup,
512 cols per band).

Key structure (vs a naive port of the reference):
  - softmax(2) -> sigmoid(d), and every sigmoid is computed as
    0.5 + 0.5*tanh(v/2) with the affine part folded into the next layer's
    weights/biases, so the only activation function used anywhere is tanh.
  - The estimator's first layer consumes a per-ST "combined tile" xc
    [96, 512] = [x (64p) | tanh(c/2) (16p) | tanh(d/2) (16p)] so the whole
    7->8 layer is ONE matmul per supertile (the sampler/conv features are
    copied into xc by cheap DVE tensor_scalar copies).
  - The sampler hidden tanh (4 units) is offloaded off the ScalarE:
    a DVE tensor_scalar drains PSUM with (+bias, max -1) and a single custom
    DVE instruction (deg-7 odd polynomial + upper clamp, 8 ALU stages)
    finishes tanh(3.2*u).  The 1/3.2 scale and the poly's leading
    coefficient fold into the A / B matmul weights.  ScalarE keeps the
    accuracy-critical tanh's (zt, h1-pre, h2-pre) exact; tau-band copies into
    xc run 3x on Pool + 1x on DVE (GPSIMD cannot touch PSUM, SBUF only).
  - E-stage outputs of a group's 2 pairs land in 64 partitions of the pa
    PSUM bank (reused late in the period), drained by one DVE add(eb3).
  - PSUM: pa/pe 1 bank, zt 1, pd 2, pcA 2, pcB 2 = 8 banks exactly.
  - Software pipeline in 16 periods; SCHEDULE gives per-stage period lags
    and per-engine queue order (tuned against TimelineSim); steady state is
    ScalarE-bound at ~4.1us/group.
"""
import numpy as np
from contextlib import ExitStack

import ml_dtypes

B_TOTAL = 4194304
N_CORES = 8
R_CORE = B_TOTAL // N_CORES        # 524288
G = 16
COLS = 512
ST_ROWS = G * COLS                  # 8192
N_ST = R_CORE // ST_ROWS            # 64
N_PAIRS = N_ST // 2                 # 32
N_G = N_PAIRS // 2                  # 16 groups (2 pairs, 4 STs)

CLAMP = 3.2                         # tanh(CLAMP)=0.9967; poly fitted on [-1,1]

BIAS_NAMES = ["ztb", "eb2b", "eb1b", "sab", "eb3b"]

POOL_BUFS = {}
RAMP_SQUEEZE = False
TAIL_SQUEEZE = False
TC_SPLIT = True
WARMUPS = 10
CP_DVE = 2

# (stage, lag): stage for group g=p-lag emitted in period p, in this order.
SCHEDULE = [
    ("dma", -2),
    ("tD", 5), ("tCA", 4), ("tCB", 4), ("zt", 2),
    ("A0", 0), ("A1", 0),
    ("cl0", 0), ("po0", 0), ("cl1", 0), ("po1", 0), ("cp", 2),
    ("D0", 4), ("D1", 4),
    ("C0", 3), ("C1", 3), ("C2", 3), ("C3", 3),
    ("E0", 5), ("E1", 5),
    ("cv0", 1), ("cv1", 1), ("B0", 1), ("B1", 1),
    ("out", 5),
]


# ---------------- poly fit (deg-7 odd, approx-minimax) ----------------

def _fit_tanh7(C):
    u = np.linspace(0, 1, 20001)
    y = np.tanh(C * u)
    A = np.stack([u ** (2 * k + 1) for k in range(4)], axis=1)
    w = np.ones_like(u)
    coef = None
    for _ in range(80):
        coef, *_ = np.linalg.lstsq(A * w[:, None], y * w, rcond=None)
        r = np.abs(A @ coef - y)
        w *= (1.0 + r / (r.max() + 1e-12)) ** 2
        w /= w.max()
    q = np.polynomial.Polynomial(coef)
    roots = q.roots()
    rr = [x for x in roots if abs(x.imag) < 1e-9]
    cc = [x for x in roots if x.imag > 1e-9]
    assert len(rr) == 1 and len(cc) == 1, roots
    r1 = float(rr[0].real)
    p1 = float(-2 * cc[0].real)
    q1 = float(abs(cc[0]) ** 2)
    k = float(coef[-1])
    return k, r1, p1, q1

POLY_K, POLY_R1, POLY_P1, POLY_Q1 = _fit_tanh7(CLAMP)


def _poly_ref(in0, in1, c0, c1, c2):
    # matches the Spec body: v = min(in0, 1); (t-c0)*((t+c1)*t+c2)*v
    v = np.minimum(np.asarray(in0, np.float32), 1.0)
    t = v * v
    return ((t - c0) * ((t + c1) * t + c2) * v).astype(np.float32)


_DVE_OP = [None]


def _get_tanh_op():
    if _DVE_OP[0] is not None:
        return _DVE_OP[0]
    from concourse.dve_spec import (
        Spec, Src0, C0, C1, C2, One, minn, sq, lower, _has_src1,
    )
    from concourse.dve_uop import DveOpSpec
    from concourse.dve_ops import DveOp, OPS, CUSTOM_DVE_SPECS, _SUB_OPCODE_FOR_NAME

    name = "TANH7_ANT_EQNN"
    if name not in _SUB_OPCODE_FOR_NAME:
        v = minn(Src0, One)
        t = sq(v)
        spec = Spec(body=((t - C0) * ((t + C1) * t + C2)) * v,
                    reference=_poly_ref)
        row = max(_SUB_OPCODE_FOR_NAME.values()) + 1
        assert row < 0x20
        _SUB_OPCODE_FOR_NAME[name] = row
        shas = {}
        for ver in ("v3", "v4"):
            s = DveOpSpec(name=name, opcode=row, uops=lower(spec, ver=ver),
                          rd1_en=_has_src1(spec))
            shas[ver] = s.sha(ver)
        op = DveOp(name, spec, subdim=False, uops_sha=shas)
        OPS.append(op)
        CUSTOM_DVE_SPECS[name] = spec
        _DVE_OP[0] = op
    else:
        from concourse.dve_ops import OPS as _ops
        _DVE_OP[0] = next(o for o in _ops if o.name == name)
    return _DVE_OP[0]


# ---------------- host-side weights ----------------

def _build_weights(conv_w, conv_b, sW1, sb1, sW2, sb2,
                   eW1, eb1, eW2, eb2, eW3, eb3):
    """All lhsT matrices [128 or 96, 128] fp32-encoded (fed as float32r)."""
    f64 = np.float64
    conv_w = np.asarray(conv_w, f64).reshape(4)
    sW1, sb1 = np.asarray(sW1, f64), np.asarray(sb1, f64)
    sW2, sb2 = np.asarray(sW2, f64), np.asarray(sb2, f64)
    eW1, eb1 = np.asarray(eW1, f64), np.asarray(eb1, f64)
    eW2, eb2 = np.asarray(eW2, f64), np.asarray(eb2, f64)
    eW3, eb3 = np.asarray(eW3, f64), np.asarray(eb3, f64)

    mats = {}

    # --- A (sampler pre-act / CLAMP): x pair tile -> pa bank.
    # pair tile partitions: 64*half + 16*f + g ; out: 64*half + 16*u + g.
    A = np.zeros((128, 128), f64)
    for h in (0, 1):
        for f in range(2):
            for u in range(4):
                wv = sW1[f, u] / CLAMP
                if wv == 0.0:
                    continue
                for g in range(G):
                    A[64 * h + 16 * f + g, 64 * h + 16 * u + g] = wv
    mats["A0"] = A          # same lhsT for both pairs
    mats["A1"] = A

    # --- CONV (pair kk): x -> zt bands; value c/2 (+bias via ACT bias).
    # zt partitions: 64*kk + 32*h + {0-15: tc, 16-31: td}.
    for kk in (0, 1):
        M = np.zeros((128, 128), f64)
        for h in (0, 1):
            for f in range(4):
                for g in range(G):
                    M[64 * h + 16 * f + g, 64 * kk + 32 * h + 0 + g] = \
                        conv_w[f] * 0.5
        mats[f"CONV{kk}"] = M
    # --- B (pair kk): samp -> zt d-band; d = dw^T tanh + db; samp holds
    # tanh/POLY_K so scale by POLY_K; also *0.5 for the sigma->tanh trick.
    dw = (sW2[:, 0] - sW2[:, 1])
    for kk in (0, 1):
        M = np.zeros((128, 128), f64)
        for h in (0, 1):
            for u in range(4):
                wv = dw[u] * POLY_K * 0.5
                for g in range(G):
                    M[64 * h + 16 * u + g, 64 * kk + 32 * h + 16 + g] = wv
        mats[f"B{kk}"] = M
    # --- C: combined tile [96,512] -> h1 pre (8 out bands, 128 partitions).
    # xc partitions: 0-63: 16f+g (x); 64-79: tc; 80-95: td.
    w_c = eW1[4]
    w_s = eW1[5] - eW1[6]
    C = np.zeros((96, 128), f64)
    for f in range(4):
        for o in range(8):
            wv = eW1[f, o]
            for g in range(G):
                C[16 * f + g, 16 * o + g] = wv
    for o in range(8):
        for g in range(G):
            C[64 + g, 16 * o + g] = 0.5 * w_c[o]
            C[80 + g, 16 * o + g] = 0.5 * w_s[o]
    mats["C"] = C

    # --- D (contract one ST's h1 [8 bands] -> h2 [4 bands]).
    # pd layout per pair: cols [0,512): lo-ST h2 at partitions 0-63,
    # hi-ST h2 at 64-127  -> two lhsT: D_lo (cols 0-63), D_hi (cols 64-127).
    for nm, col0 in (("D_lo", 0), ("D_hi", 64)):
        M = np.zeros((128, 128), f64)
        for f in range(8):
            for o in range(4):
                wv = eW2[f, o]
                for g in range(G):
                    M[16 * f + g, col0 + 16 * o + g] = wv
        mats[nm] = M

    # --- E (pair kk): h2 [128] -> out band; out partitions 32*kk+16*h+g.
    for kk in (0, 1):
        M = np.zeros((128, 128), f64)
        for h in (0, 1):
            for o in range(4):
                wv = eW3[o, 0]
                for g in range(G):
                    M[64 * h + 16 * o + g, 32 * kk + 16 * h + g] = wv
        mats[f"E{kk}"] = M

    # --- biases (fp32, [128,1] each) ---
    biases = {}
    ztb = np.zeros(128, f64)
    for kk in (0, 1):
        for h in (0, 1):
            ztb[64 * kk + 32 * h + 0:64 * kk + 32 * h + 16] = conv_b[0] * 0.5
            ztb[64 * kk + 32 * h + 16:64 * kk + 32 * h + 32] = \
                (sb2[0] - sb2[1]) * 0.5
    biases["ztb"] = ztb
    eb2b = np.zeros(128, f64)
    for h in (0, 1):
        for o in range(4):
            eb2b[64 * h + 16 * o:64 * h + 16 * o + 16] = eb2[o]
    biases["eb2b"] = eb2b
    eb1_eff = eb1 + eW1[6] + 0.5 * w_c + 0.5 * w_s
    eb1b = np.zeros(128, f64)
    for o in range(8):
        eb1b[16 * o:16 * o + 16] = eb1_eff[o]
    biases["eb1b"] = eb1b
    sab = np.zeros(128, f64)
    for h in (0, 1):
        for u in range(4):
            sab[64 * h + 16 * u:64 * h + 16 * u + 16] = sb1[u] / CLAMP
    biases["sab"] = sab
    biases["eb3b"] = np.full(128, eb3[0], f64)

    names = ["A0", "A1", "CONV0", "CONV1", "B0", "B1", "D_lo", "D_hi",
             "E0", "E1"]
    b16 = ml_dtypes.bfloat16
    Wpack = np.stack([mats[n] for n in names], axis=1)       # (128, 10, 128)
    Wpack = np.ascontiguousarray(
        Wpack.reshape(128, len(names) * 128).astype(b16))
    Cpack = np.ascontiguousarray(mats["C"].astype(b16))      # (96,128)
    Bpack = np.ascontiguousarray(np.stack(
        [biases[n] for n in BIAS_NAMES], axis=1).astype(np.float32))
    return Wpack, Cpack, Bpack, names


def _pack_inputs(x):
    """x (R_CORE,4) fp32 -> (pair tiles, xc tiles) in bf16.

    pair tile p: [128, 512], partition 64*h + 16*f + g, col c
                 = x[ST(2p+h) row g*512+c, f]
    xc tile g:   [64, 2048], partition 16*f + gg, col 512*s + c
                 = x[ST(4g+s) row gg*512+c, f]   (s = 2*kk + h)
    """
    xs = x.reshape(N_ST, G, COLS, 4)                  # [st, g, c, f]
    stf = np.ascontiguousarray(xs.transpose(0, 3, 1, 2))  # [st, f, g, c]
    stf = stf.reshape(N_ST, 64, COLS)                 # partition 16f+g
    pair = stf.reshape(N_PAIRS, 2, 64, COLS).reshape(N_PAIRS, 128, COLS)
    xc = stf.reshape(N_G, 4, 64, COLS).transpose(0, 2, 1, 3)
    xc = np.ascontiguousarray(xc.reshape(N_G, 64, 4 * COLS))
    b16 = ml_dtypes.bfloat16
    pairs2 = pair.reshape(N_G, 2, 128, COLS).transpose(0, 2, 1, 3)
    pairs2 = np.ascontiguousarray(pairs2.reshape(N_G, 128, 2 * COLS))
    return (pairs2.astype(b16), xc.astype(b16))


def _unpack_out(y):
    """y (N_G//2, 64, 1024) -> (R_CORE, 1) fp32.
    partition 32*kk + 16*h + g, col c -> ST(4g'+2kk+h) row g*512+c."""
    y = np.asarray(y, np.float32).reshape(N_G // 2, 64, 2, COLS)
    y = y.transpose(0, 2, 1, 3)                      # [g2, half, 64, c]
    y = y.reshape(N_G, 2, 2, G, COLS)                # [g,kk,h,gg,c]
    y = y.transpose(0, 1, 2, 3, 4)     # already [g, kk, h, gg, c]
    # ST index = 4g + 2kk + h ; rows = st*8192 + gg*512 + c
    return np.ascontiguousarray(y.reshape(R_CORE, 1))


# ---------------- device program ----------------

_CACHED = {}


def _build_program():
    import concourse.bacc as bacc
    import concourse.tile as tile
    from concourse import mybir

    F32 = mybir.dt.float32
    F32R = mybir.dt.float32r
    BF16 = mybir.dt.bfloat16
    AF = mybir.ActivationFunctionType
    ALU = mybir.AluOpType
    tanh_op = _get_tanh_op()

    nc = bacc.Bacc("TRN2", target_bir_lowering=False, debug=False)
    x_d = nc.dram_tensor("X", [N_G, 128, 2 * COLS], BF16,
                         kind="ExternalInput")
    xc_d = nc.dram_tensor("XC", [N_G, 64, 4 * COLS], BF16,
                          kind="ExternalInput")
    w_d = nc.dram_tensor("W", [128, 10 * 128], BF16, kind="ExternalInput")
    wc_d = nc.dram_tensor("WC", [96, 128], BF16, kind="ExternalInput")
    b_d = nc.dram_tensor("BIAS", [128, len(BIAS_NAMES)], F32,
                         kind="ExternalInput")
    y_d = nc.dram_tensor("Y", [N_G // 2, 64, 2 * COLS], BF16,
                         kind="ExternalOutput")

    WN = {n: i for i, n in enumerate(
        ["A0", "A1", "CONV0", "CONV1", "B0", "B1", "D_lo", "D_hi",
         "E0", "E1"])}
    BI = {n: i for i, n in enumerate(BIAS_NAMES)}

    with tile.TileContext(nc) as tc, ExitStack() as ctx:
        const = ctx.enter_context(tc.tile_pool(name="const", bufs=1))
        xp = ctx.enter_context(tc.tile_pool(name="xp", bufs=POOL_BUFS.get("xp", 12)))
        xcp = ctx.enter_context(tc.tile_pool(name="xcp", bufs=POOL_BUFS.get("xcp", 8)))
        sap = ctx.enter_context(tc.tile_pool(name="sap", bufs=POOL_BUFS.get("sap", 3)))
        sampp = ctx.enter_context(tc.tile_pool(name="sampp", bufs=POOL_BUFS.get("sampp", 5)))
        ztaup = ctx.enter_context(tc.tile_pool(name="ztaup", bufs=POOL_BUFS.get("ztaup", 3)))
        h1p = ctx.enter_context(tc.tile_pool(name="h1p", bufs=POOL_BUFS.get("h1p", 3)))
        h2p = ctx.enter_context(tc.tile_pool(name="h2p", bufs=POOL_BUFS.get("h2p", 3)))
        accp = ctx.enter_context(tc.tile_pool(name="accp", bufs=POOL_BUFS.get("accp", 3)))
        # PSUM: pa(1, shared with pe) + zt(1) + pd(2) + pc(4) = 8 banks
        pA = ctx.enter_context(tc.tile_pool(name="pA", bufs=1, space="PSUM"))
        pZ = ctx.enter_context(tc.tile_pool(name="pZ", bufs=1, space="PSUM"))
        pD = ctx.enter_context(tc.tile_pool(name="pD", bufs=1, space="PSUM"))
        pCA = ctx.enter_context(tc.tile_pool(name="pCA", bufs=1, space="PSUM"))
        pCB = (ctx.enter_context(tc.tile_pool(name="pCB", bufs=1,
                                              space="PSUM"))
               if TC_SPLIT else pCA)

        bt = const.tile([128, len(BIAS_NAMES)], F32)
        # Warm the ACT table (tanh only) immediately.
        warm = const.tile([128, 1], F32)
        nc.scalar.activation(warm[:], bt[:, 0:1], AF.Tanh)

        wt = const.tile([128, 10 * 128], BF16)
        wct = const.tile([96, 128], BF16)

        # PE p-state pre-warm (cost model: 3us ramp to full speed).
        # memset-backed so the warm matmuls start before any DMA lands.
        pewarm = const.tile([128, 256], BF16)
        nc.vector.memset(pewarm[:], 0.0)
        warm_ps = pCA.tile([128, 1024], F32, tag="pc0")
        for _ in range(WARMUPS):
            nc.tensor.matmul(warm_ps[:, 0:256], pewarm[:, 0:128], pewarm[:],
                             start=True, stop=True, skip_group_check=True)

        xt = {}     # pair -> x tile
        xct = {}    # group -> xc tile

        def dma_x(g, pieces=1):
            # one start for both pairs (HWDGE is ~625ns serial per start)
            t = xp.tile([128, 2 * COLS], BF16, tag="x2")
            nc.sync.dma_start(out=t[:], in_=x_d[g])
            xt[2 * g] = t[:, 0:COLS]
            xt[2 * g + 1] = t[:, COLS:2 * COLS]
            t = xcp.tile([96, 4 * COLS], BF16, tag="xc")
            nc.sync.dma_start(out=t[0:64, :], in_=xc_d[g])
            xct[g] = t

        dma_x(0)
        nc.sync.dma_start(out=wt[:, 0:256], in_=w_d[:, 0:256])
        nc.sync.dma_start(out=bt[:], in_=b_d[:])
        nc.sync.dma_start(out=wt[:, 256:640], in_=w_d[:, 256:640])
        dma_x(1)
        nc.sync.dma_start(out=wct[:], in_=wc_d[:])
        nc.sync.dma_start(out=wt[:, 640:1280], in_=w_d[:, 640:1280])
        dma_x(2)

        def W(name):
            m = WN[name]
            return wt[:, m * 128:(m + 1) * 128]

        def bias(name):
            return bt[:, BI[name]:BI[name] + 1]

        pa_t = {}   # pair -> pa psum tile
        sa_t = {}   # pair -> clamped sbuf tile
        samp = {}   # pair -> poly out (tanh/POLY_K)
        zt_t = {}   # group -> zt psum
        ztau = {}   # group -> tanh(zt) sbuf  [tc|td bands]
        pc_t = {}   # (group, half) -> pc psum [128, 1024]
        pd_t = {}   # group -> pd psum [128, 1024] (pair0|pair1)
        h1 = {}     # (group, half) -> h1 sbuf [128, 1024]
        h2 = {}     # group -> h2 sbuf [128, 1024]
        pe_t = {}   # group -> pe psum (pa pool, 64 partitions)

        def emit_A(g, kk):
            p = 2 * g + kk
            pa = pA.tile([128, COLS], F32, tag="pa")
            pa_t[p] = pa
            nc.tensor.matmul(pa[:], W(f"A{kk}"), xt[p][:],
                             start=True, stop=True, skip_group_check=True)

        def emit_clamp(g, kk):
            # DVE: sa = max(pa + sab, -1.0)  (upper clamp inside the DVE op;
            # GPSIMD cannot read PSUM, so this drain must be DVE)
            p = 2 * g + kk
            t = sap.tile([128, COLS], BF16, tag="sa")
            sa_t[p] = t
            nc.vector.tensor_scalar(t[:], pa_t[p][:], bias("sab"), -1.0,
                                    ALU.add, ALU.max)
            del pa_t[p]

        def emit_poly(g, kk):
            p = 2 * g + kk
            t = sampp.tile([128, COLS], BF16, tag="samp")
            samp[p] = t
            nc.vector._custom_dve(tanh_op, out=t[:], in0=sa_t[p][:],
                                  s0=POLY_R1, s1=POLY_P1, imm2=POLY_Q1)
            del sa_t[p]

        def emit_conv(g, kk):
            p = 2 * g + kk
            if kk == 0:
                zt = pZ.tile([128, COLS], F32, tag="pz")
                zt_t[g] = zt
            nc.tensor.matmul(zt_t[g][:], W(f"CONV{kk}"), xt[p][:],
                             start=(kk == 0), stop=False,
                             skip_group_check=True)
            del xt[p]
            # underlying [128,1024] tile is freed once both halves are deleted

        def emit_B(g, kk):
            nc.tensor.matmul(zt_t[g][:], W(f"B{kk}"), samp[2 * g + kk][:],
                             start=False, stop=(kk == 1),
                             skip_group_check=True)
            del samp[2 * g + kk]

        def emit_tanh_zt(g):
            t = ztaup.tile([128, COLS], BF16, tag="ztau")
            ztau[g] = t
            nc.scalar.activation(t[:], zt_t[g][:], AF.Tanh, bias=bias("ztb"))
            del zt_t[g]

        def emit_copies(g):
            # 4 copies [32,512]: ztau bands -> xc partitions 64-95.
            # 3 on Pool (SBUF->SBUF is legal there), 1 on DVE for balance.
            for s in range(4):
                eng = nc.vector if s >= 4 - CP_DVE else nc.gpsimd
                eng.tensor_scalar(
                    xct[g][64:96, s * COLS:(s + 1) * COLS],
                    ztau[g][32 * s:32 * s + 32, :], 1.0, None, ALU.mult)
            del ztau[g]

        def emit_C(g, s, alloc=False):
            if not TC_SPLIT:
                if s == 0:
                    pct = pCA.tile([128, 4 * COLS], F32, tag="pc0")
                    pc_t[(g, 0)] = pc_t[(g, 1)] = pct
                nc.tensor.matmul(pc_t[(g, 0)][:, s * COLS:(s + 1) * COLS],
                                 wct[:], xct[g][:, s * COLS:(s + 1) * COLS],
                                 start=True, stop=True,
                                 skip_group_check=True)
                if s == 3:
                    del xct[g]
                return
            hh = s // 2
            if s % 2 == 0:
                pool = pCA if hh == 0 else pCB
                pct = pool.tile([128, 2 * COLS], F32, tag=f"pc{hh}")
                pc_t[(g, hh)] = pct
            nc.tensor.matmul(pc_t[(g, hh)][:, (s % 2) * COLS:
                                           (s % 2 + 1) * COLS],
                             wct[:], xct[g][:, s * COLS:(s + 1) * COLS],
                             start=True, stop=True, skip_group_check=True)
            if s == 3:
                del xct[g]

        def emit_tanhC(g, hh):
            if not TC_SPLIT:
                if hh == 1:
                    return
                t = h1p.tile([128, 4 * COLS], BF16, tag="h1")
                h1[(g, 0)] = t[:, 0:2 * COLS]
                h1[(g, 1)] = t[:, 2 * COLS:4 * COLS]
                nc.scalar.activation(t[:], pc_t[(g, 0)][:], AF.Tanh,
                                     bias=bias("eb1b"))
                del pc_t[(g, 0)], pc_t[(g, 1)]
                return
            t = h1p.tile([128, 2 * COLS], BF16, tag="h1")
            h1[(g, hh)] = t
            nc.scalar.activation(t[:], pc_t[(g, hh)][:], AF.Tanh,
                                 bias=bias("eb1b"))
            del pc_t[(g, hh)]

        def emit_D(g, kk):
            if kk == 0:
                pd = pD.tile([128, 2 * COLS], F32, tag="pd")
                pd_t[g] = pd
            nc.tensor.matmul(pd_t[g][:, kk * COLS:(kk + 1) * COLS],
                             W("D_lo"), h1[(g, kk)][:, 0:COLS],
                             start=True, stop=False, skip_group_check=True)
            nc.tensor.matmul(pd_t[g][:, kk * COLS:(kk + 1) * COLS],
                             W("D_hi"), h1[(g, kk)][:, COLS:2 * COLS],
                             start=False, stop=True, skip_group_check=True)
            del h1[(g, kk)]

        def emit_tanhD(g):
            t = h2p.tile([128, 2 * COLS], BF16, tag="h2")
            h2[g] = t
            nc.scalar.activation(t[:], pd_t[g][:], AF.Tanh, bias=bias("eb2b"))
            del pd_t[g]

        def emit_E(g, kk):
            if kk == 0:
                pe = pA.tile([128, COLS], F32, tag="pa")
                pe_t[g] = pe
            nc.tensor.matmul(pe_t[g][0:64, :], W(f"E{kk}")[:, 0:64],
                             h2[g][:, kk * COLS:(kk + 1) * COLS],
                             start=(kk == 0), stop=(kk == 1),
                             skip_group_check=True)
            if kk == 1:
                del h2[g]

        acc_t = {}

        def emit_out(g):
            if g % 2 == 0:
                acc = accp.tile([64, 2 * COLS], BF16, tag="acc")
                acc_t[g // 2] = acc
            acc = acc_t[g // 2]
            nc.vector.tensor_scalar(acc[:, (g % 2) * COLS:(g % 2 + 1) * COLS],
                                    pe_t[g][0:64, :],
                                    bt[0:64, BI["eb3b"]:BI["eb3b"] + 1], None,
                                    ALU.add)
            if g % 2 == 1:
                nc.sync.dma_start(out=y_d[g // 2], in_=acc[:])
                del acc_t[g // 2]
            del pe_t[g]

        # ---- software pipeline (emission order = per-engine queue order) ----
        stages = {
            "A0": lambda g: emit_A(g, 0),
            "A1": lambda g: emit_A(g, 1),
            "cl0": lambda g: emit_clamp(g, 0),
            "cl1": lambda g: emit_clamp(g, 1),
            "po0": lambda g: emit_poly(g, 0),
            "po1": lambda g: emit_poly(g, 1),
            "cv0": lambda g: emit_conv(g, 0),
            "cv1": lambda g: emit_conv(g, 1),
            "B0": lambda g: emit_B(g, 0),
            "B1": lambda g: emit_B(g, 1),
            "zt": emit_tanh_zt,
            "cp": emit_copies,
            "C0": lambda g: emit_C(g, 0, alloc=True),
            "C1": lambda g: emit_C(g, 1),
            "C2": lambda g: emit_C(g, 2),
            "C3": lambda g: emit_C(g, 3),
            "tCA": lambda g: emit_tanhC(g, 0),
            "tCB": lambda g: emit_tanhC(g, 1),
            "D0": lambda g: emit_D(g, 0),
            "D1": lambda g: emit_D(g, 1),
            "tD": emit_tanhD,
            "E0": lambda g: emit_E(g, 0),
            "E1": lambda g: emit_E(g, 1),
            "out": emit_out,
        }
        # shallow (dependency-minimal) lag per stage for ramp/tail squeeze
        SHALLOW = {"A0": 0, "A1": 0, "cl0": 0, "cl1": 0, "po0": 0, "po1": 0,
                   "cv0": 0, "cv1": 0, "B0": 0, "B1": 0, "zt": 1, "cp": 1,
                   "C0": 1, "C1": 1, "C2": 1, "C3": 1, "tCA": 2, "tCB": 2,
                   "D0": 2, "D1": 2, "tD": 3, "E0": 3, "E1": 3, "out": 3}

        def lag_eff(name, lag, g):
            if name == "dma":
                return lag
            s = SHALLOW[name]
            if RAMP_SQUEEZE and TAIL_SQUEEZE:
                return min(lag, s + g, s + (N_G - 1 - g))
            if RAMP_SQUEEZE:
                return min(lag, s + g)
            if TAIL_SQUEEZE:
                return min(lag, s + (N_G - 1 - g))
            return lag

        max_lag = max(lag for _, lag in SCHEDULE)
        for p in range(N_G + max_lag + 1):
            for name, lag in SCHEDULE:
                if name == "dma":
                    g = p - lag
                    if 3 <= g < N_G:
                        dma_x(g)
                    continue
                for g in range(N_G):
                    if g + lag_eff(name, lag, g) == p:
                        stages[name](g)

    nc.compile()
    return nc


def kernel(**inputs):
    from concourse.bass_utils import run_bass_kernel_spmd

    inputs = {k: np.asarray(v, np.float32) for k, v in inputs.items()}
    x = inputs["inputs"]
    Wpack, Cpack, Bpack, _ = _build_weights(
        inputs["conv_w"], inputs["conv_b"], inputs["sW1"], inputs["sb1"],
        inputs["sW2"], inputs["sb2"], inputs["eW1"], inputs["eb1"],
        inputs["eW2"], inputs["eb2"], inputs["eW3"], inputs["eb3"])

    if "nc" not in _CACHED:
        _CACHED["nc"] = _build_program()
    nc = _CACHED["nc"]

    in_maps = []
    for c in range(N_CORES):
        xc = x[c * R_CORE:(c + 1) * R_CORE]
        pair, xcomb = _pack_inputs(xc)
        in_maps.append({"X": pair, "XC": xcomb, "W": Wpack, "WC": Cpack,
                        "BIAS": Bpack})

    res = run_bass_kernel_spmd(nc, in_maps, list(range(N_CORES)))
    out = np.concatenate(
        [_unpack_out(res.results[c]["Y"]) for c in range(N_CORES)], axis=0)
    return out.astype(np.float32)


# revision 49
# speedup vs baseline: 1.2777x; 1.0024x over previous
"""Trainium2 Bass kernel for nn_EstimatorQNNGen104 (dense tiny-MLP over 4.2M rows).

Pure data parallel over 8 NeuronCores (R_core = 524288 rows/core), bf16 data +
fp32r weights.  Per core the batch is processed in 16 "groups" of 2 pairs
(4 supertiles of 8192 rows; banding: partition = 16*feature + rowgroup,
512 cols per band).

Key structure (vs a naive port of the reference):
  - softmax(2) -> sigmoid(d), and every sigmoid is computed as
    0.5 + 0.5*tanh(v/2) with the affine part folded into the next layer's
    weights/biases, so the only activation function used anywhere is tanh.
  - The estimator's first layer consumes a per-ST "combined tile" xc
    [96, 512] = [x (64p) | tanh(c/2) (16p) | tanh(d/2) (16p)] so the whole
    7->8 layer is ONE matmul per supertile (the sampler/conv features are
    copied into xc by cheap DVE tensor_scalar copies).
  - The sampler hidden tanh (4 units) is offloaded off the ScalarE:
    a DVE tensor_scalar drains PSUM with (+bias, max -1) and a single custom
    DVE instruction (deg-7 odd polynomial + upper clamp, 8 ALU stages)
    finishes tanh(3.2*u).  The 1/3.2 scale and the poly's leading
    coefficient fold into the A / B matmul weights.  ScalarE keeps the
    accuracy-critical tanh's (zt, h1-pre, h2-pre) exact; tau-band copies into
    xc run 3x on Pool + 1x on DVE (GPSIMD cannot touch PSUM, SBUF only).
  - E-stage outputs of a group's 2 pairs land in 64 partitions of the pa
    PSUM bank (reused late in the period), drained by one DVE add(eb3).
  - PSUM: pa/pe 1 bank, zt 1, pd 2, pcA 2, pcB 2 = 8 banks exactly.
  - Software pipeline in 16 periods; SCHEDULE gives per-stage period lags
    and per-engine queue order (tuned against TimelineSim); steady state is
    ScalarE-bound at ~4.1us/group.
"""
import numpy as np
from contextlib import ExitStack

import ml_dtypes

B_TOTAL = 4194304
N_CORES = 8
R_CORE = B_TOTAL // N_CORES        # 524288
G = 16
COLS = 512
ST_ROWS = G * COLS                  # 8192
N_ST = R_CORE // ST_ROWS            # 64
N_PAIRS = N_ST // 2                 # 32
N_G = N_PAIRS // 2                  # 16 groups (2 pairs, 4 STs)

CLAMP = 3.2                         # tanh(CLAMP)=0.9967; poly fitted on [-1,1]

BIAS_NAMES = ["ztb", "eb2b", "eb1b", "sab", "eb3b"]

POOL_BUFS = {}
RAMP_SQUEEZE = False
TAIL_SQUEEZE = False
TC_SPLIT = True
WARMUPS = 10
CP_DVE = 2

# (stage, lag): stage for group g=p-lag emitted in period p, in this order.
SCHEDULE = [
    ("dma", -2),
    ("tD", 5), ("tCA", 4), ("tCB", 4), ("zt", 2),
    ("A0", 0), ("A1", 0),
    ("cl0", 0), ("po0", 0), ("cl1", 0), ("po1", 0), ("cp", 2),
    ("D0", 4), ("D1", 4),
    ("C0", 3), ("C1", 3), ("C2", 3), ("C3", 3),
    ("E0", 5), ("E1", 5),
    ("cv0", 1), ("cv1", 1), ("B0", 1), ("B1", 1),
    ("out", 5),
]


# ---------------- poly fit (deg-7 odd, approx-minimax) ----------------

def _fit_tanh7(C):
    u = np.linspace(0, 1, 20001)
    y = np.tanh(C * u)
    A = np.stack([u ** (2 * k + 1) for k in range(4)], axis=1)
    w = np.ones_like(u)
    coef = None
    for _ in range(80):
        coef, *_ = np.linalg.lstsq(A * w[:, None], y * w, rcond=None)
        r = np.abs(A @ coef - y)
        w *= (1.0 + r / (r.max() + 1e-12)) ** 2
        w /= w.max()
    q = np.polynomial.Polynomial(coef)
    roots = q.roots()
    rr = [x for x in roots if abs(x.imag) < 1e-9]
    cc = [x for x in roots if x.imag > 1e-9]
    assert len(rr) == 1 and len(cc) == 1, roots
    r1 = float(rr[0].real)
    p1 = float(-2 * cc[0].real)
    q1 = float(abs(cc[0]) ** 2)
    k = float(coef[-1])
    return k, r1, p1, q1

POLY_K, POLY_R1, POLY_P1, POLY_Q1 = _fit_tanh7(CLAMP)


def _poly_ref(in0, in1, c0, c1, c2):
    # matches the Spec body: v = min(in0, 1); (t-c0)*((t+c1)*t+c2)*v
    v = np.minimum(np.asarray(in0, np.float32), 1.0)
    t = v * v
    return ((t - c0) * ((t + c1) * t + c2) * v).astype(np.float32)


_DVE_OP = [None]


def _get_tanh_op():
    if _DVE_OP[0] is not None:
        return _DVE_OP[0]
    from concourse.dve_spec import (
        Spec, Src0, C0, C1, C2, One, minn, sq, lower, _has_src1,
    )
    from concourse.dve_uop import DveOpSpec
    from concourse.dve_ops import DveOp, OPS, CUSTOM_DVE_SPECS, _SUB_OPCODE_FOR_NAME

    name = "TANH7_ANT_EQNN"
    if name not in _SUB_OPCODE_FOR_NAME:
        v = minn(Src0, One)
        t = sq(v)
        spec = Spec(body=((t - C0) * ((t + C1) * t + C2)) * v,
                    reference=_poly_ref)
        row = max(_SUB_OPCODE_FOR_NAME.values()) + 1
        assert row < 0x20
        _SUB_OPCODE_FOR_NAME[name] = row
        shas = {}
        for ver in ("v3", "v4"):
            s = DveOpSpec(name=name, opcode=row, uops=lower(spec, ver=ver),
                          rd1_en=_has_src1(spec))
            shas[ver] = s.sha(ver)
        op = DveOp(name, spec, subdim=False, uops_sha=shas)
        OPS.append(op)
        CUSTOM_DVE_SPECS[name] = spec
        _DVE_OP[0] = op
    else:
        from concourse.dve_ops import OPS as _ops
        _DVE_OP[0] = next(o for o in _ops if o.name == name)
    return _DVE_OP[0]


# ---------------- host-side weights ----------------

def _build_weights(conv_w, conv_b, sW1, sb1, sW2, sb2,
                   eW1, eb1, eW2, eb2, eW3, eb3):
    """All lhsT matrices [128 or 96, 128] fp32-encoded (fed as float32r)."""
    f64 = np.float64
    conv_w = np.asarray(conv_w, f64).reshape(4)
    sW1, sb1 = np.asarray(sW1, f64), np.asarray(sb1, f64)
    sW2, sb2 = np.asarray(sW2, f64), np.asarray(sb2, f64)
    eW1, eb1 = np.asarray(eW1, f64), np.asarray(eb1, f64)
    eW2, eb2 = np.asarray(eW2, f64), np.asarray(eb2, f64)
    eW3, eb3 = np.asarray(eW3, f64), np.asarray(eb3, f64)

    mats = {}

    # --- A (sampler pre-act / CLAMP): x pair tile -> pa bank.
    # pair tile partitions: 64*half + 16*f + g ; out: 64*half + 16*u + g.
    A = np.zeros((128, 128), f64)
    for h in (0, 1):
        for f in range(2):
            for u in range(4):
                wv = sW1[f, u] / CLAMP
                if wv == 0.0:
                    continue
                for g in range(G):
                    A[64 * h + 16 * f + g, 64 * h + 16 * u + g] = wv
    mats["A0"] = A          # same lhsT for both pairs
    mats["A1"] = A

    # --- CONV (pair kk): x -> zt bands; value c/2 (+bias via ACT bias).
    # zt partitions: 64*kk + 32*h + {0-15: tc, 16-31: td}.
    for kk in (0, 1):
        M = np.zeros((128, 128), f64)
        for h in (0, 1):
            for f in range(4):
                for g in range(G):
                    M[64 * h + 16 * f + g, 64 * kk + 32 * h + 0 + g] = \
                        conv_w[f] * 0.5
        mats[f"CONV{kk}"] = M
    # --- B (pair kk): samp -> zt d-band; d = dw^T tanh + db; samp holds
    # tanh/POLY_K so scale by POLY_K; also *0.5 for the sigma->tanh trick.
    dw = (sW2[:, 0] - sW2[:, 1])
    for kk in (0, 1):
        M = np.zeros((128, 128), f64)
        for h in (0, 1):
            for u in range(4):
                wv = dw[u] * POLY_K * 0.5
                for g in range(G):
                    M[64 * h + 16 * u + g, 64 * kk + 32 * h + 16 + g] = wv
        mats[f"B{kk}"] = M
    # --- C: combined tile [96,512] -> h1 pre (8 out bands, 128 partitions).
    # xc partitions: 0-63: 16f+g (x); 64-79: tc; 80-95: td.
    w_c = eW1[4]
    w_s = eW1[5] - eW1[6]
    C = np.zeros((96, 128), f64)
    for f in range(4):
        for o in range(8):
            wv = eW1[f, o]
            for g in range(G):
                C[16 * f + g, 16 * o + g] = wv
    for o in range(8):
        for g in range(G):
            C[64 + g, 16 * o + g] = 0.5 * w_c[o]
            C[80 + g, 16 * o + g] = 0.5 * w_s[o]
    mats["C"] = C

    # --- D (contract one ST's h1 [8 bands] -> h2 [4 bands]).
    # pd layout per pair: cols [0,512): lo-ST h2 at partitions 0-63,
    # hi-ST h2 at 64-127  -> two lhsT: D_lo (cols 0-63), D_hi (cols 64-127).
    for nm, col0 in (("D_lo", 0), ("D_hi", 64)):
        M = np.zeros((128, 128), f64)
        for f in range(8):
            for o in range(4):
                wv = eW2[f, o]
                for g in range(G):
                    M[16 * f + g, col0 + 16 * o + g] = wv
        mats[nm] = M

    # --- E (pair kk): h2 [128] -> out band; out partitions 32*kk+16*h+g.
    for kk in (0, 1):
        M = np.zeros((128, 128), f64)
        for h in (0, 1):
            for o in range(4):
                wv = eW3[o, 0]
                for g in range(G):
                    M[64 * h + 16 * o + g, 32 * kk + 16 * h + g] = wv
        mats[f"E{kk}"] = M

    # --- biases (fp32, [128,1] each) ---
    biases = {}
    ztb = np.zeros(128, f64)
    for kk in (0, 1):
        for h in (0, 1):
            ztb[64 * kk + 32 * h + 0:64 * kk + 32 * h + 16] = conv_b[0] * 0.5
            ztb[64 * kk + 32 * h + 16:64 * kk + 32 * h + 32] = \
                (sb2[0] - sb2[1]) * 0.5
    biases["ztb"] = ztb
    eb2b = np.zeros(128, f64)
    for h in (0, 1):
        for o in range(4):
            eb2b[64 * h + 16 * o:64 * h + 16 * o + 16] = eb2[o]
    biases["eb2b"] = eb2b
    eb1_eff = eb1 + eW1[6] + 0.5 * w_c + 0.5 * w_s
    eb1b = np.zeros(128, f64)
    for o in range(8):
        eb1b[16 * o:16 * o + 16] = eb1_eff[o]
    biases["eb1b"] = eb1b
    sab = np.zeros(128, f64)
    for h in (0, 1):
        for u in range(4):
            sab[64 * h + 16 * u:64 * h + 16 * u + 16] = sb1[u] / CLAMP
    biases["sab"] = sab
    biases["eb3b"] = np.full(128, eb3[0], f64)

    names = ["A0", "A1", "CONV0", "CONV1", "B0", "B1", "D_lo", "D_hi",
             "E0", "E1"]
    b16 = ml_dtypes.bfloat16
    Wpack = np.stack([mats[n] for n in names], axis=1)       # (128, 10, 128)
    Wpack = np.ascontiguousarray(
        Wpack.reshape(128, len(names) * 128).astype(b16))
    Cpack = np.ascontiguousarray(mats["C"].astype(b16))      # (96,128)
    Bpack = np.ascontiguousarray(np.stack(
        [biases[n] for n in BIAS_NAMES], axis=1).astype(np.float32))
    return Wpack, Cpack, Bpack, names


def _pack_inputs(x):
    """x (R_CORE,4) fp32 -> (pair tiles, xc tiles) in bf16.

    pair tile p: [128, 512], partition 64*h + 16*f + g, col c
                 = x[ST(2p+h) row g*512+c, f]
    xc tile g:   [64, 2048], partition 16*f + gg, col 512*s + c
                 = x[ST(4g+s) row gg*512+c, f]   (s = 2*kk + h)
    """
    xs = x.reshape(N_ST, G, COLS, 4)                  # [st, g, c, f]
    stf = np.ascontiguousarray(xs.transpose(0, 3, 1, 2))  # [st, f, g, c]
    stf = stf.reshape(N_ST, 64, COLS)                 # partition 16f+g
    pair = stf.reshape(N_PAIRS, 2, 64, COLS).reshape(N_PAIRS, 128, COLS)
    xc = stf.reshape(N_G, 4, 64, COLS).transpose(0, 2, 1, 3)
    xc = np.ascontiguousarray(xc.reshape(N_G, 64, 4 * COLS))
    b16 = ml_dtypes.bfloat16
    pairs2 = pair.reshape(N_G, 2, 128, COLS).transpose(0, 2, 1, 3)
    pairs2 = np.ascontiguousarray(pairs2.reshape(N_G, 128, 2 * COLS))
    return (pairs2.astype(b16), xc.astype(b16))


def _unpack_out(y):
    """y (N_G, 64, 512) -> (R_CORE, 1) fp32.
    partition 32*kk + 16*h + g, col c -> ST(4g'+2kk+h) row g*512+c."""
    y = np.asarray(y, np.float32).reshape(N_G, 2, 2, G, COLS)  # [g,kk,h,gg,c]
    y = y.transpose(0, 1, 2, 3, 4)     # already [g, kk, h, gg, c]
    # ST index = 4g + 2kk + h ; rows = st*8192 + gg*512 + c
    return np.ascontiguousarray(y.reshape(R_CORE, 1))


# ---------------- device program ----------------

_CACHED = {}


def _build_program():
    import concourse.bacc as bacc
    import concourse.tile as tile
    from concourse import mybir

    F32 = mybir.dt.float32
    F32R = mybir.dt.float32r
    BF16 = mybir.dt.bfloat16
    AF = mybir.ActivationFunctionType
    ALU = mybir.AluOpType
    tanh_op = _get_tanh_op()

    nc = bacc.Bacc("TRN2", target_bir_lowering=False, debug=False)
    x_d = nc.dram_tensor("X", [N_G, 128, 2 * COLS], BF16,
                         kind="ExternalInput")
    xc_d = nc.dram_tensor("XC", [N_G, 64, 4 * COLS], BF16,
                          kind="ExternalInput")
    w_d = nc.dram_tensor("W", [128, 10 * 128], BF16, kind="ExternalInput")
    wc_d = nc.dram_tensor("WC", [96, 128], BF16, kind="ExternalInput")
    b_d = nc.dram_tensor("BIAS", [128, len(BIAS_NAMES)], F32,
                         kind="ExternalInput")
    y_d = nc.dram_tensor("Y", [N_G, 64, COLS], BF16, kind="ExternalOutput")

    WN = {n: i for i, n in enumerate(
        ["A0", "A1", "CONV0", "CONV1", "B0", "B1", "D_lo", "D_hi",
         "E0", "E1"])}
    BI = {n: i for i, n in enumerate(BIAS_NAMES)}

    with tile.TileContext(nc) as tc, ExitStack() as ctx:
        const = ctx.enter_context(tc.tile_pool(name="const", bufs=1))
        xp = ctx.enter_context(tc.tile_pool(name="xp", bufs=POOL_BUFS.get("xp", 12)))
        xcp = ctx.enter_context(tc.tile_pool(name="xcp", bufs=POOL_BUFS.get("xcp", 8)))
        sap = ctx.enter_context(tc.tile_pool(name="sap", bufs=POOL_BUFS.get("sap", 3)))
        sampp = ctx.enter_context(tc.tile_pool(name="sampp", bufs=POOL_BUFS.get("sampp", 5)))
        ztaup = ctx.enter_context(tc.tile_pool(name="ztaup", bufs=POOL_BUFS.get("ztaup", 3)))
        h1p = ctx.enter_context(tc.tile_pool(name="h1p", bufs=POOL_BUFS.get("h1p", 3)))
        h2p = ctx.enter_context(tc.tile_pool(name="h2p", bufs=POOL_BUFS.get("h2p", 3)))
        accp = ctx.enter_context(tc.tile_pool(name="accp", bufs=POOL_BUFS.get("accp", 3)))
        # PSUM: pa(1, shared with pe) + zt(1) + pd(2) + pc(4) = 8 banks
        pA = ctx.enter_context(tc.tile_pool(name="pA", bufs=1, space="PSUM"))
        pZ = ctx.enter_context(tc.tile_pool(name="pZ", bufs=1, space="PSUM"))
        pD = ctx.enter_context(tc.tile_pool(name="pD", bufs=1, space="PSUM"))
        pCA = ctx.enter_context(tc.tile_pool(name="pCA", bufs=1, space="PSUM"))
        pCB = (ctx.enter_context(tc.tile_pool(name="pCB", bufs=1,
                                              space="PSUM"))
               if TC_SPLIT else pCA)

        bt = const.tile([128, len(BIAS_NAMES)], F32)
        # Warm the ACT table (tanh only) immediately.
        warm = const.tile([128, 1], F32)
        nc.scalar.activation(warm[:], bt[:, 0:1], AF.Tanh)

        wt = const.tile([128, 10 * 128], BF16)
        wct = const.tile([96, 128], BF16)

        # PE p-state pre-warm (cost model: 3us ramp to full speed).
        # memset-backed so the warm matmuls start before any DMA lands.
        pewarm = const.tile([128, 256], BF16)
        nc.vector.memset(pewarm[:], 0.0)
        warm_ps = pCA.tile([128, 1024], F32, tag="pc0")
        for _ in range(WARMUPS):
            nc.tensor.matmul(warm_ps[:, 0:256], pewarm[:, 0:128], pewarm[:],
                             start=True, stop=True, skip_group_check=True)

        xt = {}     # pair -> x tile
        xct = {}    # group -> xc tile

        def dma_x(g, pieces=1):
            # one start for both pairs (HWDGE is ~625ns serial per start)
            t = xp.tile([128, 2 * COLS], BF16, tag="x2")
            nc.sync.dma_start(out=t[:], in_=x_d[g])
            xt[2 * g] = t[:, 0:COLS]
            xt[2 * g + 1] = t[:, COLS:2 * COLS]
            t = xcp.tile([96, 4 * COLS], BF16, tag="xc")
            nc.sync.dma_start(out=t[0:64, :], in_=xc_d[g])
            xct[g] = t

        dma_x(0)
        nc.sync.dma_start(out=wt[:, 0:256], in_=w_d[:, 0:256])
        nc.sync.dma_start(out=bt[:], in_=b_d[:])
        nc.sync.dma_start(out=wt[:, 256:640], in_=w_d[:, 256:640])
        dma_x(1)
        nc.sync.dma_start(out=wct[:], in_=wc_d[:])
        nc.sync.dma_start(out=wt[:, 640:1280], in_=w_d[:, 640:1280])
        dma_x(2)

        def W(name):
            m = WN[name]
            return wt[:, m * 128:(m + 1) * 128]

        def bias(name):
            return bt[:, BI[name]:BI[name] + 1]

        pa_t = {}   # pair -> pa psum tile
        sa_t = {}   # pair -> clamped sbuf tile
        samp = {}   # pair -> poly out (tanh/POLY_K)
        zt_t = {}   # group -> zt psum
        ztau = {}   # group -> tanh(zt) sbuf  [tc|td bands]
        pc_t = {}   # (group, half) -> pc psum [128, 1024]
        pd_t = {}   # group -> pd psum [128, 1024] (pair0|pair1)
        h1 = {}     # (group, half) -> h1 sbuf [128, 1024]
        h2 = {}     # group -> h2 sbuf [128, 1024]
        pe_t = {}   # group -> pe psum (pa pool, 64 partitions)

        def emit_A(g, kk):
            p = 2 * g + kk
            pa = pA.tile([128, COLS], F32, tag="pa")
            pa_t[p] = pa
            nc.tensor.matmul(pa[:], W(f"A{kk}"), xt[p][:],
                             start=True, stop=True, skip_group_check=True)

        def emit_clamp(g, kk):
            # DVE: sa = max(pa + sab, -1.0)  (upper clamp inside the DVE op;
            # GPSIMD cannot read PSUM, so this drain must be DVE)
            p = 2 * g + kk
            t = sap.tile([128, COLS], BF16, tag="sa")
            sa_t[p] = t
            nc.vector.tensor_scalar(t[:], pa_t[p][:], bias("sab"), -1.0,
                                    ALU.add, ALU.max)
            del pa_t[p]

        def emit_poly(g, kk):
            p = 2 * g + kk
            t = sampp.tile([128, COLS], BF16, tag="samp")
            samp[p] = t
            nc.vector._custom_dve(tanh_op, out=t[:], in0=sa_t[p][:],
                                  s0=POLY_R1, s1=POLY_P1, imm2=POLY_Q1)
            del sa_t[p]

        def emit_conv(g, kk):
            p = 2 * g + kk
            if kk == 0:
                zt = pZ.tile([128, COLS], F32, tag="pz")
                zt_t[g] = zt
            nc.tensor.matmul(zt_t[g][:], W(f"CONV{kk}"), xt[p][:],
                             start=(kk == 0), stop=False,
                             skip_group_check=True)
            del xt[p]
            # underlying [128,1024] tile is freed once both halves are deleted

        def emit_B(g, kk):
            nc.tensor.matmul(zt_t[g][:], W(f"B{kk}"), samp[2 * g + kk][:],
                             start=False, stop=(kk == 1),
                             skip_group_check=True)
            del samp[2 * g + kk]

        def emit_tanh_zt(g):
            t = ztaup.tile([128, COLS], BF16, tag="ztau")
            ztau[g] = t
            nc.scalar.activation(t[:], zt_t[g][:], AF.Tanh, bias=bias("ztb"))
            del zt_t[g]

        def emit_copies(g):
            # 4 copies [32,512]: ztau bands -> xc partitions 64-95.
            # 3 on Pool (SBUF->SBUF is legal there), 1 on DVE for balance.
            for s in range(4):
                eng = nc.vector if s >= 4 - CP_DVE else nc.gpsimd
                eng.tensor_scalar(
                    xct[g][64:96, s * COLS:(s + 1) * COLS],
                    ztau[g][32 * s:32 * s + 32, :], 1.0, None, ALU.mult)
            del ztau[g]

        def emit_C(g, s, alloc=False):
            if not TC_SPLIT:
                if s == 0:
                    pct = pCA.tile([128, 4 * COLS], F32, tag="pc0")
                    pc_t[(g, 0)] = pc_t[(g, 1)] = pct
                nc.tensor.matmul(pc_t[(g, 0)][:, s * COLS:(s + 1) * COLS],
                                 wct[:], xct[g][:, s * COLS:(s + 1) * COLS],
                                 start=True, stop=True,
                                 skip_group_check=True)
                if s == 3:
                    del xct[g]
                return
            hh = s // 2
            if s % 2 == 0:
                pool = pCA if hh == 0 else pCB
                pct = pool.tile([128, 2 * COLS], F32, tag=f"pc{hh}")
                pc_t[(g, hh)] = pct
            nc.tensor.matmul(pc_t[(g, hh)][:, (s % 2) * COLS:
                                           (s % 2 + 1) * COLS],
                             wct[:], xct[g][:, s * COLS:(s + 1) * COLS],
                             start=True, stop=True, skip_group_check=True)
            if s == 3:
                del xct[g]

        def emit_tanhC(g, hh):
            if not TC_SPLIT:
                if hh == 1:
                    return
                t = h1p.tile([128, 4 * COLS], BF16, tag="h1")
                h1[(g, 0)] = t[:, 0:2 * COLS]
                h1[(g, 1)] = t[:, 2 * COLS:4 * COLS]
                nc.scalar.activation(t[:], pc_t[(g, 0)][:], AF.Tanh,
                                     bias=bias("eb1b"))
                del pc_t[(g, 0)], pc_t[(g, 1)]
                return
            t = h1p.tile([128, 2 * COLS], BF16, tag="h1")
            h1[(g, hh)] = t
            nc.scalar.activation(t[:], pc_t[(g, hh)][:], AF.Tanh,
                                 bias=bias("eb1b"))
            del pc_t[(g, hh)]

        def emit_D(g, kk):
            if kk == 0:
                pd = pD.tile([128, 2 * COLS], F32, tag="pd")
                pd_t[g] = pd
            nc.tensor.matmul(pd_t[g][:, kk * COLS:(kk + 1) * COLS],
                             W("D_lo"), h1[(g, kk)][:, 0:COLS],
                             start=True, stop=False, skip_group_check=True)
            nc.tensor.matmul(pd_t[g][:, kk * COLS:(kk + 1) * COLS],
                             W("D_hi"), h1[(g, kk)][:, COLS:2 * COLS],
                             start=False, stop=True, skip_group_check=True)
            del h1[(g, kk)]

        def emit_tanhD(g):
            t = h2p.tile([128, 2 * COLS], BF16, tag="h2")
            h2[g] = t
            nc.scalar.activation(t[:], pd_t[g][:], AF.Tanh, bias=bias("eb2b"))
            del pd_t[g]

        def emit_E(g, kk):
            if kk == 0:
                pe = pA.tile([128, COLS], F32, tag="pa")
                pe_t[g] = pe
            nc.tensor.matmul(pe_t[g][0:64, :], W(f"E{kk}")[:, 0:64],
                             h2[g][:, kk * COLS:(kk + 1) * COLS],
                             start=(kk == 0), stop=(kk == 1),
                             skip_group_check=True)
            if kk == 1:
                del h2[g]

        def emit_out(g):
            acc = accp.tile([64, COLS], BF16, tag="acc")
            nc.vector.tensor_scalar(acc[:], pe_t[g][0:64, :],
                                    bt[0:64, BI["eb3b"]:BI["eb3b"] + 1], None,
                                    ALU.add)
            nc.sync.dma_start(out=y_d[g], in_=acc[:])
            del pe_t[g]

        # ---- software pipeline (emission order = per-engine queue order) ----
        stages = {
            "A0": lambda g: emit_A(g, 0),
            "A1": lambda g: emit_A(g, 1),
            "cl0": lambda g: emit_clamp(g, 0),
            "cl1": lambda g: emit_clamp(g, 1),
            "po0": lambda g: emit_poly(g, 0),
            "po1": lambda g: emit_poly(g, 1),
            "cv0": lambda g: emit_conv(g, 0),
            "cv1": lambda g: emit_conv(g, 1),
            "B0": lambda g: emit_B(g, 0),
            "B1": lambda g: emit_B(g, 1),
            "zt": emit_tanh_zt,
            "cp": emit_copies,
            "C0": lambda g: emit_C(g, 0, alloc=True),
            "C1": lambda g: emit_C(g, 1),
            "C2": lambda g: emit_C(g, 2),
            "C3": lambda g: emit_C(g, 3),
            "tCA": lambda g: emit_tanhC(g, 0),
            "tCB": lambda g: emit_tanhC(g, 1),
            "D0": lambda g: emit_D(g, 0),
            "D1": lambda g: emit_D(g, 1),
            "tD": emit_tanhD,
            "E0": lambda g: emit_E(g, 0),
            "E1": lambda g: emit_E(g, 1),
            "out": emit_out,
        }
        # shallow (dependency-minimal) lag per stage for ramp/tail squeeze
        SHALLOW = {"A0": 0, "A1": 0, "cl0": 0, "cl1": 0, "po0": 0, "po1": 0,
                   "cv0": 0, "cv1": 0, "B0": 0, "B1": 0, "zt": 1, "cp": 1,
                   "C0": 1, "C1": 1, "C2": 1, "C3": 1, "tCA": 2, "tCB": 2,
                   "D0": 2, "D1": 2, "tD": 3, "E0": 3, "E1": 3, "out": 3}

        def lag_eff(name, lag, g):
            if name == "dma":
                return lag
            s = SHALLOW[name]
            if RAMP_SQUEEZE and TAIL_SQUEEZE:
                return min(lag, s + g, s + (N_G - 1 - g))
            if RAMP_SQUEEZE:
                return min(lag, s + g)
            if TAIL_SQUEEZE:
                return min(lag, s + (N_G - 1 - g))
            return lag

        max_lag = max(lag for _, lag in SCHEDULE)
        for p in range(N_G + max_lag + 1):
            for name, lag in SCHEDULE:
                if name == "dma":
                    g = p - lag
                    if 3 <= g < N_G:
                        dma_x(g)
                    continue
                for g in range(N_G):
                    if g + lag_eff(name, lag, g) == p:
                        stages[name](g)

    nc.compile()
    return nc


def kernel(**inputs):
    from concourse.bass_utils import run_bass_kernel_spmd

    inputs = {k: np.asarray(v, np.float32) for k, v in inputs.items()}
    x = inputs["inputs"]
    Wpack, Cpack, Bpack, _ = _build_weights(
        inputs["conv_w"], inputs["conv_b"], inputs["sW1"], inputs["sb1"],
        inputs["sW2"], inputs["sb2"], inputs["eW1"], inputs["eb1"],
        inputs["eW2"], inputs["eb2"], inputs["eW3"], inputs["eb3"])

    if "nc" not in _CACHED:
        _CACHED["nc"] = _build_program()
    nc = _CACHED["nc"]

    in_maps = []
    for c in range(N_CORES):
        xc = x[c * R_CORE:(c + 1) * R_CORE]
        pair, xcomb = _pack_inputs(xc)
        in_maps.append({"X": pair, "XC": xcomb, "W": Wpack, "WC": Cpack,
                        "BIAS": Bpack})

    res = run_bass_kernel_spmd(nc, in_maps, list(range(N_CORES)))
    out = np.concatenate(
        [_unpack_out(res.results[c]["Y"]) for c in range(N_CORES)], axis=0)
    return out.astype(np.float32)
